# revision 21
# baseline (speedup 1.0000x reference)
"""Trainium2 Bass kernel for nn_DualGRUDecoder (scatter_memory).

Sharding: stage 1 data-parallel over batch (128 -> 16 rows/core) for
attention + GRU + copy-attention; all-gather of the GRU/attention state;
stage 2 vocab-parallel (50000 -> 6250 cols/core) for the fc projection,
softmax (cross-core stats exchange), and the scatter_add of copy
probabilities (done on-chip via gpsimd local_scatter with f32-as-u16
pairs). Host assembles: concat vocab shards (axis=1) + batch shards of
h_new (axis=0).
"""

import numpy as np
import ml_dtypes

import contextlib

import concourse.bacc as bacc
import concourse.bass as bass
import concourse.mybir as mybir
import concourse.tile as tile
from concourse import bass_utils

F32 = mybir.dt.float32
BF16 = mybir.dt.bfloat16
I16 = mybir.dt.int16
U16 = mybir.dt.uint16
BF = ml_dtypes.bfloat16

NCORES = 8
B, BC = 128, 16
S1, S2 = 200, 200
S = S1 + S2                  # 400
EMB, DEC, E2 = 256, 512, 1024
F = DEC + E2                 # 1536 state dim
XF = EMB + E2                # 1280 GRU input dim
V, VC, VP = 50000, 6250, 6272
PW, NPIECE = 896, 7          # scatter piece width (f32), 7*896 = 6272
NEG = -1e10

JROWS = S * BC               # 6400 joint-attention rows
CROWS = S1 * BC              # 3200 copy-attention rows
NTW = 400                    # energy N-tile width (25 s-values)
HALF = 3200                  # score half size (200 s-values)


DBG = False
NO_CC = False
NO_SCAT = False
NO_ENERGY = False
NO_FC = False
SMALL_OUT = False
STAGE1_ONLY = False


def build_nc(R):
    nc = bacc.Bacc("TRN2", target_bir_lowering=False, debug=False)
    nc.num_devices = NCORES

    def din(name, shape, dt):
        return nc.dram_tensor(name, shape, dt, kind="ExternalInput")

    # per-core activations
    enc_t_d = din("encT", [E2, JROWS], BF16)
    enc_b_d = din("encB", [8, S, 128, 16], BF16)
    ht_f_d = din("hTf", [DEC, BC], F32)
    ht_b_d = din("hTb", [DEC, BC], BF16)
    emb_t_d = din("embT", [EMB, BC], BF16)
    biasj_d = din("biasJ", [BC, S], F32)
    biasc_d = din("biasC", [BC, S1], F32)
    # replicated weights
    awe_d = din("aWeT", [E2, DEC], BF16)
    awh_d = din("aWhT", [DEC, DEC], BF16)
    ab_d = din("ab", [DEC, 1], F32)
    av_d = din("av", [DEC, 1], BF16)
    cwe_d = din("cWeT", [E2, DEC], BF16)
    cwh_d = din("cWhT", [DEC, DEC], BF16)
    cb_d = din("cb", [DEC, 1], F32)
    cv_d = din("cv", [DEC, 1], BF16)
    wih_d = din("WihT", [XF, 3 * DEC], BF16)
    whh_d = din("WhhT", [DEC, 3 * DEC], BF16)
    brz_d = din("brz", [2 * DEC, 1], F32)
    bhn_d = din("bhn", [DEC, 1], F32)
    bin_d = din("bin", [DEC, 1], F32)
    gw_d = din("gw", [XF, 1], BF16)
    gb_d = din("gb", [1, 1], F32)
    # vocab shard
    fcw_d = din("fcWT", [F, VP], BF16)
    fcb_d = din("fcb", [1, VP], BF16)
    idx_d = din("idxT", [NPIECE * R * 128, 512], I16)

    out_v = nc.dram_tensor("out_v", [B, 64 if SMALL_OUT else VP], F32, kind="ExternalOutput")
    out_h = nc.dram_tensor("out_h", [BC, DEC], F32, kind="ExternalOutput")
    if DBG:
        dbg_sc = nc.dram_tensor("dbg_sc", [16, S], F32, kind="ExternalOutput")
        dbg_a = nc.dram_tensor("dbg_a", [16, S], F32, kind="ExternalOutput")
        dbg_wt = nc.dram_tensor("dbg_wt", [128, 128], F32, kind="ExternalOutput")
        dbg_hn = nc.dram_tensor("dbg_hn", [128, 64], F32, kind="ExternalOutput")
        dbg_st2 = nc.dram_tensor("dbg_st2", [128, 1536], F32, kind="ExternalOutput")
        dbg_lg = nc.dram_tensor("dbg_lg", [128, 512], F32, kind="ExternalOutput")
        dbg_adf = nc.dram_tensor("dbg_adf", [128, 200], F32, kind="ExternalOutput")
        dbg_al = nc.dram_tensor("dbg_al", [128, 4], F32, kind="ExternalOutput")

    ag1_in = nc.dram_tensor("ag1_in", [F * BC], BF16)
    ag1_out = nc.dram_tensor("ag1_out", [NCORES * F * BC], BF16,
                             addr_space="Shared")
    ag2_in = nc.dram_tensor("ag2_in", [BC * 201], F32)
    ag2_out = nc.dram_tensor("ag2_out", [NCORES * BC * 201], F32,
                             addr_space="Shared")
    ag3_in = nc.dram_tensor("ag3_in", [B * 2], F32)
    ag3_out = nc.dram_tensor("ag3_out", [NCORES * B * 2], F32,
                             addr_space="Shared")
    scr_h = [nc.dram_tensor(f"scr_h{i}", [HALF], F32) for i in range(3)]
    scr_pg = nc.dram_tensor("scr_pg", [16], F32)
    scr_w = nc.dram_tensor("scr_w", [E2 * 16], F32)
    rg = [list(range(NCORES))]

    with tile.TileContext(nc) as tc, \
         tc.tile_pool(name="wp", bufs=1) as wp, \
         tc.tile_pool(name="s1", bufs=1) as s1, \
         tc.tile_pool(name="s2", bufs=3) as s2, \
         tc.tile_pool(name="sco", bufs=1) as sco, \
         tc.tile_pool(name="pmm", bufs=4, space="PSUM") as pmm, \
         tc.tile_pool(name="psm", bufs=2, space="PSUM") as psm, \
         tc.tile_pool(name="pw", bufs=2, space="PSUM") as pw:

        def energy_scores(we_sb, htb_sb, vv_sb, nhalves, tw):
            """Scores with b-major encT: one batch row per tile of width tw.
            htb_sb [128, 4, 16] = (W_h^T h + bias) per (d, b), fused into the
            tanh bias. Returns sc [16, 400] f32 (b, s)."""
            tph = HALF // tw          # tiles (= batch rows) per half
            sc = s1.tile([16, S], F32, tag="sc")
            for h in range(nhalves):
                half = sco.tile([1, HALF], F32, tag="scores")
                for ti in range(tph):
                    bg = h * tph + ti
                    n0 = ti * tw
                    et = s2.tile([128, 8, 512], BF16, tag="big_t")
                    nc.sync.dma_start(
                        et[:, 0:8, 0:tw],
                        enc_t_d[:, bg * 400: bg * 400 + tw].rearrange(
                            "(k p) n -> p k n", p=128))
                    th = s2.tile([128, 4, 400], BF16, tag="th_t")
                    for mc in range(4):
                        e_ps = pmm.tile([128, 512], F32, tag="mm")
                        for k in range(8):
                            nc.tensor.matmul(
                                e_ps[:, :tw],
                                we_sb[:, k, mc * 128:(mc + 1) * 128],
                                et[:, k, :tw],
                                start=(k == 0), stop=(k == 7))
                        nc.scalar.activation(
                            th[:, mc, :tw], e_ps[:, :tw],
                            mybir.ActivationFunctionType.Tanh,
                            bias=htb_sb[:, mc, bg:bg + 1])
                    sc_ps = psm.tile([1, 400], F32, tag="sm")
                    for mc in range(4):
                        nc.tensor.matmul(sc_ps[:, :tw], vv_sb[:, mc, :],
                                         th[:, mc, :tw],
                                         start=(mc == 0), stop=(mc == 3))
                    nc.scalar.copy(half[0:1, n0:n0 + tw], sc_ps[:, :tw])
                hd = scr_h[h if nhalves == 2 else 2]
                nc.sync.dma_start(hd[:].rearrange("(o n) -> o n", o=1), half[:])
                nc.sync.dma_start(
                    sc[h * tph:(h + 1) * tph, :tw],
                    hd[:].rearrange("(b s) -> b s", b=tph))
            return sc

        def softmax_rows(sc, ns, bias_d):
            bb = s1.tile([16, S], F32, tag="bias")
            nc.sync.dma_start(bb[:, :ns], bias_d[:])
            nc.vector.tensor_add(sc[:, :ns], sc[:, :ns], bb[:, :ns])
            mx = s1.tile([16, 1], F32, tag="mx")
            nc.vector.reduce_max(mx[:], sc[:, :ns], axis=mybir.AxisListType.X)
            nmx = s1.tile([16, 1], F32, tag="nmx")
            nc.vector.tensor_scalar_mul(nmx[:], mx[:], -1.0)
            ex = s1.tile([16, S], F32, tag="ex")
            nc.scalar.activation(ex[:, :ns], sc[:, :ns],
                                 mybir.ActivationFunctionType.Exp,
                                 bias=nmx[:])
            sm = s1.tile([16, 1], F32, tag="sm1")
            nc.vector.reduce_sum(sm[:], ex[:, :ns], axis=mybir.AxisListType.X)
            si = s1.tile([16, 1], F32, tag="si")
            nc.vector.reciprocal(si[:], sm[:])
            a = s1.tile([16, S], F32, tag="a")
            nc.vector.tensor_scalar_mul(a[:, :ns], ex[:, :ns], si[:])
            return a

        # ---------- persistent weight loads ----------
        awe = wp.tile([128, 8, 512], BF16, tag="we")
        nc.sync.dma_start(awe[:], awe_d[:].rearrange("(k p) m -> p k m", p=128))
        awh = wp.tile([128, 4, 512], BF16, tag="wh")
        nc.sync.dma_start(awh[:], awh_d[:].rearrange("(k p) m -> p k m", p=128))
        vj = wp.tile([128, 4, 1], BF16, tag="vj")
        nc.sync.dma_start(vj[:], av_d[:].rearrange("(k p) o -> p k o", p=128))
        vc = wp.tile([128, 4, 1], BF16, tag="vc")
        nc.sync.dma_start(vc[:], cv_d[:].rearrange("(k p) o -> p k o", p=128))
        ab = wp.tile([128, 4, 1], F32, tag="ab")
        nc.sync.dma_start(ab[:], ab_d[:].rearrange("(k p) o -> p k o", p=128))
        cb = wp.tile([128, 4, 1], F32, tag="cb")
        nc.sync.dma_start(cb[:], cb_d[:].rearrange("(k p) o -> p k o", p=128))
        brz = wp.tile([128, 8, 1], F32, tag="brz")
        nc.sync.dma_start(brz[:], brz_d[:].rearrange("(k p) o -> p k o", p=128))
        bhn = wp.tile([128, 4, 1], F32, tag="bhn")
        nc.sync.dma_start(bhn[:], bhn_d[:].rearrange("(k p) o -> p k o", p=128))
        bin_ = wp.tile([128, 4, 1], F32, tag="bin")
        nc.sync.dma_start(bin_[:], bin_d[:].rearrange("(k p) o -> p k o", p=128))
        gw = wp.tile([128, 10, 1], BF16, tag="gw")
        nc.sync.dma_start(gw[:], gw_d[:].rearrange("(k p) o -> p k o", p=128))
        gb = wp.tile([1, 1], F32, tag="gb")
        nc.sync.dma_start(gb[:], gb_d[:])
        hTb = wp.tile([128, 4, 16], BF16, tag="hTb")
        nc.sync.dma_start(hTb[:], ht_b_d[:].rearrange("(k p) b -> p k b", p=128))
        hTf = wp.tile([128, 4, 16], F32, tag="hTf")
        nc.sync.dma_start(hTf[:], ht_f_d[:].rearrange("(k p) b -> p k b", p=128))
        ones1 = wp.tile([1, 128], BF16, tag="ones1")
        nc.vector.memset(ones1[:], 1.0)
        onesP = wp.tile([128, 1], BF16, tag="onesP")
        nc.vector.memset(onesP[:], 1.0)

        # xt = GRU input x^T [128, 10, 16]: chunks 0:2 emb, 2:10 weighted
        xt = wp.tile([128, 10, 16], BF16, tag="xt")
        nc.sync.dma_start(xt[:, 0:2, :],
                          emb_t_d[:].rearrange("(k p) b -> p k b", p=128))
        # st = state^T payload [128, 12, 16]: 0:4 h_new, 4:12 weighted
        st = wp.tile([128, 12, 16], BF16, tag="st")
        logits = wp.tile([128, VP], F32, tag="logits")

        with tc.tile_pool(name="gp", bufs=1) as gp:
            wih = gp.tile([128, 10, 1536], BF16, tag="wih")
            nc.sync.dma_start(wih[:],
                              wih_d[:].rearrange("(k p) m -> p k m", p=128))
            whh = gp.tile([128, 4, 1536], BF16, tag="whh")
            nc.sync.dma_start(whh[:],
                              whh_d[:].rearrange("(k p) m -> p k m", p=128))

            # ---------- joint attention ----------
            htj = s1.tile([128, 4, 16], F32, tag="htj")
            for mc in range(4):
                hp = psm.tile([128, 16], F32, tag="sm")
                for k in range(4):
                    nc.tensor.matmul(
                        hp[:], awh[:, k, mc * 128:(mc + 1) * 128],
                        hTb[:, k, :], start=(k == 0), stop=(k == 3))
                nc.scalar.activation(htj[:, mc, :], hp[:],
                                     mybir.ActivationFunctionType.Identity,
                                     bias=ab[:, mc, :])

            sc_j = energy_scores(awe, htj, vj, 2, 400)
            if DBG:
                nc.sync.dma_start(dbg_sc[:], sc_j[:])
            a_j = softmax_rows(sc_j, S, biasj_d)
            if DBG:
                nc.sync.dma_start(dbg_a[:], a_j[:])

            # transpose a_j -> aT [128, 4, 32] bf16 (s on partitions)
            a32 = s1.tile([32, 416], BF16, tag="a32")
            nc.vector.memset(a32[:], 0.0)
            nc.vector.tensor_copy(a32[:16, :S], a_j[:, :S])
            aT = s1.tile([128, 4, 32], BF16, tag="aT")
            for k in range(4):
                nj = 4 if k < 3 else 1
                for j32 in range(nj):
                    nc.vector.transpose(
                        aT[j32 * 32:(j32 + 1) * 32, k, 0:32],
                        a32[0:32, k * 128 + j32 * 32: k * 128 + (j32 + 1) * 32])

            # ---------- weighted sum: wT[e, b] = sum_s a[b,s] enc[s,b,e] ----------
            wT = s1.tile([128, 8, 16], F32, tag="wT")
            wd = scr_w
            for ec in range(32):
                c8, e0 = ec // 4, (ec % 4) * 32
                wps = pw.tile([1, 512], F32, tag="wps")
                for k in range(4):
                    sl = 128 if k < 3 else 16
                    nB = s2.tile([128, 512], BF16, tag="nB")
                    nc.sync.dma_start(
                        nB[:sl, :],
                        enc_b_d[c8, k * 128: k * 128 + sl,
                                e0:e0 + 32].rearrange("s e j -> s (e j)"))
                    tmp = s2.tile([128, 512], BF16, tag="wtmp")
                    nc.vector.tensor_mul(
                        tmp[:sl, :].rearrange("p (e j) -> p e j", j=16),
                        nB[:sl, :].rearrange("p (e j) -> p e j", j=16),
                        aT[:sl, k, None, 0:16].broadcast_to([sl, 32, 16]))
                    nc.tensor.matmul(
                        wps[0:1, :], onesP[:sl, :], tmp[:sl, :],
                        start=(k == 0), stop=(k == 3))
                wst = s2.tile([1, 512], F32, tag="wst")
                nc.scalar.copy(wst[:], wps[:])
                nc.sync.dma_start(
                    wd[ec * 512:(ec + 1) * 512].rearrange("(o n) -> o n", o=1),
                    wst[:])
            nc.sync.dma_start(
                wT[:], wd[:].rearrange("(c p j) -> p c j", p=128, j=16))
            if DBG:
                nc.sync.dma_start(dbg_wt[:], wT[:].rearrange("p c j -> p (c j)"))
            nc.vector.tensor_copy(xt[:, 2:10, :], wT[:])
            nc.vector.tensor_copy(st[:, 4:12, :], wT[:])

            # ---------- p_gen ----------
            pgp = psm.tile([1, 16], F32, tag="sm")
            for k in range(10):
                nc.tensor.matmul(pgp[:], gw[:, k, :], xt[:, k, :],
                                 start=(k == 0), stop=(k == 9))
            pgT = s1.tile([1, 16], F32, tag="pgT")
            nc.scalar.activation(pgT[:], pgp[:],
                                 mybir.ActivationFunctionType.Sigmoid,
                                 bias=gb[:])
            pgd = scr_pg
            nc.sync.dma_start(pgd[:].rearrange("(o n) -> o n", o=1), pgT[:])
            pg_b = s1.tile([16, 1], F32, tag="pg_b")
            nc.sync.dma_start(pg_b[:], pgd[:].rearrange("(b o) -> b o", o=1))

            # ---------- GRU ----------
            rz = s1.tile([128, 8, 16], F32, tag="rz")
            hnewT = s1.tile([128, 4, 16], F32, tag="hnewT")
            for mc in range(8):
                gpsum = psm.tile([128, 16], F32, tag="sm")
                for k in range(10):
                    nc.tensor.matmul(
                        gpsum[:], wih[:, k, mc * 128:(mc + 1) * 128],
                        xt[:, k, :], start=(k == 0), stop=False)
                for k in range(4):
                    nc.tensor.matmul(
                        gpsum[:], whh[:, k, mc * 128:(mc + 1) * 128],
                        hTb[:, k, :], start=False, stop=(k == 3))
                nc.scalar.activation(rz[:, mc, :], gpsum[:],
                                     mybir.ActivationFunctionType.Sigmoid,
                                     bias=brz[:, mc, :])
            for mc in range(4):
                m8 = mc + 8
                gx = psm.tile([128, 16], F32, tag="sm")
                for k in range(10):
                    nc.tensor.matmul(
                        gx[:], wih[:, k, m8 * 128:(m8 + 1) * 128],
                        xt[:, k, :], start=(k == 0), stop=(k == 9))
                gh = psm.tile([128, 16], F32, tag="sm")
                for k in range(4):
                    nc.tensor.matmul(
                        gh[:], whh[:, k, m8 * 128:(m8 + 1) * 128],
                        hTb[:, k, :], start=(k == 0), stop=(k == 3))
                hnb = s1.tile([128, 16], F32, tag="hnb")
                nc.scalar.activation(hnb[:], gh[:],
                                     mybir.ActivationFunctionType.Identity,
                                     bias=bhn[:, mc, :])
                rhn = s1.tile([128, 16], F32, tag="rhn")
                nc.vector.tensor_mul(rhn[:], rz[:, mc, :], hnb[:])
                xnr = s1.tile([128, 16], F32, tag="xnr")
                nc.vector.tensor_add(xnr[:], gx[:], rhn[:])
                n_sb = s1.tile([128, 16], F32, tag="n_sb")
                nc.scalar.activation(n_sb[:], xnr[:],
                                     mybir.ActivationFunctionType.Tanh,
                                     bias=bin_[:, mc, :])
                dh = s1.tile([128, 16], F32, tag="dh")
                nc.vector.tensor_sub(dh[:], hTf[:, mc, :], n_sb[:])
                zd = s1.tile([128, 16], F32, tag="zd")
                nc.vector.tensor_mul(zd[:], rz[:, mc + 4, :], dh[:])
                nc.vector.tensor_add(hnewT[:, mc, :], n_sb[:], zd[:])
                nc.vector.tensor_copy(st[:, mc, :], hnewT[:, mc, :])

        if DBG:
            nc.sync.dma_start(dbg_hn[:], hnewT[:].rearrange("p c j -> p (c j)"))
        # h_new natural layout -> out_h
        hp32 = s1.tile([128, 32], F32, tag="hp32")
        nc.vector.memset(hp32[:], 0.0)
        hnat = s1.tile([32, DEC], F32, tag="hnat")
        for mc in range(4):
            nc.vector.tensor_copy(hp32[:, 0:16], hnewT[:, mc, :])
            for pb in range(4):
                nc.vector.transpose(
                    hnat[0:32, mc * 128 + pb * 32: mc * 128 + (pb + 1) * 32],
                    hp32[pb * 32:(pb + 1) * 32, 0:32])
        nc.sync.dma_start(out_h[:], hnat[:16, :])

        # ---------- AG1: state ----------
        nc.sync.dma_start(
            ag1_in[:].rearrange("(p c j) -> p (c j)", p=128, j=16), st[:])
        if NO_CC:
            nc.sync.dma_start(ag1_out[0:F * BC], ag1_in[:])
        else:
            nc.gpsimd.collective_compute(
                "AllGather", mybir.AluOpType.bypass, replica_groups=rg,
                ins=[ag1_in[:]], outs=[ag1_out[:]])

        # stage-2 pools: reuse the SBUF freed by the GRU-weight pool
        es = contextlib.ExitStack()
        s3 = es.enter_context(tc.tile_pool(name="s3", bufs=1))
        s4 = es.enter_context(tc.tile_pool(name="s4", bufs=2))
        delta = s3.tile([128, VP], F32, tag="delta")

        # ---------- copy attention ----------
        cwe = wp.tile([128, 8, 512], BF16, tag="we")
        nc.sync.dma_start(cwe[:], cwe_d[:].rearrange("(k p) m -> p k m", p=128))
        cwh = wp.tile([128, 4, 512], BF16, tag="wh")
        nc.sync.dma_start(cwh[:], cwh_d[:].rearrange("(k p) m -> p k m", p=128))
        htc = s1.tile([128, 4, 16], F32, tag="htc")
        for mc in range(4):
            hp = psm.tile([128, 16], F32, tag="sm")
            for k in range(4):
                nc.tensor.matmul(
                    hp[:], cwh[:, k, mc * 128:(mc + 1) * 128],
                    st[:, k, :], start=(k == 0), stop=(k == 3))
            nc.scalar.activation(htc[:, mc, :], hp[:],
                                 mybir.ActivationFunctionType.Identity,
                                 bias=cb[:, mc, :])
        sc_c = energy_scores(cwe, htc, vc, 1, 200)
        a_c = softmax_rows(sc_c, S1, biasc_d)
        adp = s1.tile([16, 201], F32, tag="adp")
        apg = s1.tile([16, S1], F32, tag="apg")
        nc.vector.tensor_scalar_mul(apg[:], a_c[:, :S1], pg_b[:])
        nc.vector.tensor_sub(adp[:, 0:S1], a_c[:, :S1], apg[:])
        nc.vector.tensor_copy(adp[:, 200:201], pg_b[:])
        nc.sync.dma_start(
            ag2_in[:].rearrange("(b c) -> b c", b=16), adp[:])
        if NO_CC:
            nc.sync.dma_start(ag2_out[0:BC * 201], ag2_in[:])
        else:
            nc.gpsimd.collective_compute(
                "AllGather", mybir.AluOpType.bypass, replica_groups=rg,
                ins=[ag2_in[:]], outs=[ag2_out[:]])

        # ---------- stage 2: fc matmul over vocab shard ----------
        st2 = s1.tile([128, 12, 128], BF16, tag="st2")
        nc.sync.dma_start(
            st2[:], ag1_out[:].rearrange("(r p c j) -> p c r j",
                                         r=NCORES, p=128, j=16))
        if DBG:
            st2f = s1.tile([128, 12, 128], F32, tag="st2f")
            nc.vector.tensor_copy(st2f[:], st2[:])
            nc.sync.dma_start(dbg_st2[:], st2f[:].rearrange("p c j -> p (c j)"))
        if NO_FC:
            nc.vector.memset(logits[:], 0.001)
        v0 = 0
        while v0 < (VP if not NO_FC else 0):
            nv = min(512, VP - v0)
            fw = s4.tile([128, 12, 512], BF16, tag="fw")
            nc.sync.dma_start(
                fw[:, :, :nv],
                fcw_d[:, v0:v0 + nv].rearrange("(k p) n -> p k n", p=128))
            fcbt = s2.tile([1, 512], BF16, tag="fcb_t")
            nc.sync.dma_start(fcbt[0:1, :nv], fcb_d[0:1, v0:v0 + nv])
            lp = pmm.tile([128, 512], F32, tag="mm")
            for k in range(12):
                nc.tensor.matmul(lp[:, :nv], st2[:, k, :], fw[:, k, :nv],
                                 start=(k == 0), stop=False)
            nc.tensor.matmul(lp[:, :nv], ones1[:], fcbt[0:1, :nv],
                             start=False, stop=True)
            nc.scalar.copy(logits[:, v0:v0 + nv], lp[:, :nv])
            v0 += nv

        if DBG:
            nc.sync.dma_start(dbg_lg[:], logits[:, 0:512])
        m_c = s1.tile([128, 1], F32, tag="m_c")
        nc.vector.reduce_max(m_c[:], logits[:], axis=mybir.AxisListType.X)
        nmc = s1.tile([128, 1], F32, tag="nmc")
        nc.vector.tensor_scalar_mul(nmc[:], m_c[:], -1.0)
        nc.scalar.activation(logits[:], logits[:],
                             mybir.ActivationFunctionType.Exp, bias=nmc[:])
        s_c = s1.tile([128, 1], F32, tag="s_c")
        nc.vector.reduce_sum(s_c[:], logits[:], axis=mybir.AxisListType.X)

        stats = s1.tile([128, 2], F32, tag="stats")
        nc.vector.tensor_copy(stats[:, 0:1], m_c[:])
        nc.vector.tensor_copy(stats[:, 1:2], s_c[:])
        nc.sync.dma_start(
            ag3_in[:].rearrange("(p c) -> p c", p=128), stats[:])
        if NO_CC:
            nc.sync.dma_start(ag3_out[0:B * 2], ag3_in[:])
        else:
            nc.gpsimd.collective_compute(
                "AllGather", mybir.AluOpType.bypass, replica_groups=rg,
                ins=[ag3_in[:]], outs=[ag3_out[:]])

        mg = s1.tile([128, 8, 2], F32, tag="mg")
        nc.sync.dma_start(
            mg[:], ag3_out[:].rearrange("(r p c) -> p r c", r=NCORES, p=128))
        M = s1.tile([128, 1], F32, tag="M")
        nc.vector.reduce_max(M[:], mg[:, :, 0:1], axis=mybir.AxisListType.XY)
        nM = s1.tile([128, 1], F32, tag="nM")
        nc.vector.tensor_scalar_mul(nM[:], M[:], -1.0)
        eR = s1.tile([128, 8], F32, tag="eR")
        nc.scalar.activation(eR[:], mg[:, :, 0],
                             mybir.ActivationFunctionType.Exp, bias=nM[:])
        t8 = s1.tile([128, 8], F32, tag="t8")
        nc.vector.tensor_mul(t8[:], eR[:], mg[:, :, 1])
        Ssum = s1.tile([128, 1], F32, tag="Ssum")
        nc.vector.reduce_sum(Ssum[:], t8[:], axis=mybir.AxisListType.X)
        Sinv = s1.tile([128, 1], F32, tag="Sinv")
        nc.vector.reciprocal(Sinv[:], Ssum[:])
        emd = s1.tile([128, 1], F32, tag="emd")
        nc.scalar.activation(emd[:], m_c[:],
                             mybir.ActivationFunctionType.Exp, bias=nM[:])
        pgf = s1.tile([128, 1], F32, tag="pgf")
        nc.sync.dma_start(
            pgf[:], ag2_out[:].rearrange("(b c) -> b c", c=201)[:, 200:201])
        alpha = s1.tile([128, 1], F32, tag="alpha")
        nc.vector.tensor_mul(alpha[:], emd[:], Sinv[:])
        nc.vector.tensor_mul(alpha[:], alpha[:], pgf[:])

        # ---------- scatter_add of copy probabilities ----------
        adf = s1.tile([128, 200], F32, tag="adf")
        nc.sync.dma_start(
            adf[:], ag2_out[:].rearrange("(b c) -> b c", c=201)[:, 0:200])
        if DBG:
            nc.sync.dma_start(dbg_adf[:], adf[:])
            dal = s1.tile([128, 4], F32, tag="dal")
            nc.vector.tensor_copy(dal[:, 0:1], alpha[:])
            nc.vector.tensor_copy(dal[:, 1:2], m_c[:])
            nc.vector.tensor_copy(dal[:, 2:3], s_c[:])
            nc.vector.tensor_copy(dal[:, 3:4], Ssum[:])
            nc.sync.dma_start(dbg_al[:], dal[:])
        for pc in range(NPIECE if not NO_SCAT else 0):
            for r in range(R):
                ix = s2.tile([128, 512], I16, tag="idx_t")
                nc.sync.dma_start(
                    ix[:],
                    idx_d[(pc * R + r) * 128:(pc * R + r + 1) * 128, :])
                dl = s2.tile([128, PW], F32, tag="dl")
                nc.gpsimd.local_scatter(
                    dl[:].bitcast(U16), adf[:].bitcast(U16), ix[:, 0:400],
                    channels=128, num_elems=2 * PW, num_idxs=400)
                if r == 0:
                    nc.vector.tensor_copy(delta[:, pc * PW:(pc + 1) * PW], dl[:])
                else:
                    nc.vector.tensor_add(delta[:, pc * PW:(pc + 1) * PW],
                                         delta[:, pc * PW:(pc + 1) * PW],
                                         dl[:])
        nc.vector.scalar_tensor_tensor(
            out=logits[:], in0=logits[:], scalar=alpha[:], in1=delta[:],
            op0=mybir.AluOpType.mult, op1=mybir.AluOpType.add)
        nc.sync.dma_start(out_v[:], logits[:, 0:64] if SMALL_OUT else logits[:])
        es.close()

    nc.compile()
    return nc


def _prep(inputs):
    """Host-side sharding/layout prep. Returns (in_maps, R)."""
    inp = {k: np.asarray(v) for k, v in inputs.items()}
    e1, e2 = inp["encoder_outputs1"], inp["encoder_outputs2"]
    src1 = inp["src1"].astype(np.int64)
    emb_all = inp["embedding"][inp["input"].astype(np.int64)]  # [B, EMB]
    attn_W, attn_b, attn_v = inp["attn_W"], inp["attn_b"], inp["attn_v"]
    copy_W, copy_b, copy_v = inp["copy_W"], inp["copy_b"], inp["copy_v"]
    mask_j = np.concatenate([inp["mask1"], inp["mask2"]], axis=1)
    mask_c = inp["mask1"] * inp["triple_mask"]
    bias_j_all = np.where(mask_j == 0, NEG, 0.0).astype(np.float32)
    bias_c_all = np.where(mask_c == 0, NEG, 0.0).astype(np.float32)

    shared = {
        "aWeT": attn_W[DEC:].astype(BF), "aWhT": np.ascontiguousarray(attn_W[:DEC]).astype(BF),
        "ab": attn_b[:, None].astype(np.float32), "av": attn_v[:, None].astype(BF),
        "cWeT": copy_W[DEC:].astype(BF), "cWhT": np.ascontiguousarray(copy_W[:DEC]).astype(BF),
        "cb": copy_b[:, None].astype(np.float32), "cv": copy_v[:, None].astype(BF),
        "WihT": np.ascontiguousarray(inp["gru_Wih"].T).astype(BF),
        "WhhT": np.ascontiguousarray(inp["gru_Whh"].T).astype(BF),
        "brz": (inp["gru_bih"] + inp["gru_bhh"])[:2 * DEC, None].astype(np.float32),
        "bhn": inp["gru_bhh"][2 * DEC:, None].astype(np.float32),
        "bin": inp["gru_bih"][2 * DEC:, None].astype(np.float32),
        "gw": np.ascontiguousarray(
            np.concatenate([inp["gate_W"][0, E2:], inp["gate_W"][0, :E2]])[:, None]).astype(BF),
        "gb": inp["gate_b"].reshape(1, 1).astype(np.float32),
    }

    # scatter plan: occurrence rounds
    occ = {}
    rmax = 1
    entry = [[] for _ in range(NCORES)]
    for s in range(S1):
        for b in range(B):
            v = int(src1[s, b])
            c = min(v // VC, NCORES - 1)
            vl = v - c * VC
            key = (b, v)
            r = occ.get(key, 0)
            occ[key] = r + 1
            rmax = max(rmax, r + 1)
            entry[c].append((b, s, vl, r))
    R = rmax

    in_maps = []
    for c in range(NCORES):
        rows = slice(c * BC, (c + 1) * BC)
        enc = np.concatenate([e1[:, rows], e2[:, rows]], axis=0).astype(BF)
        encT = np.ascontiguousarray(enc.transpose(2, 1, 0)).reshape(E2, JROWS)
        encB = np.ascontiguousarray(
            enc.reshape(S, BC, 8, 128).transpose(2, 0, 3, 1))
        hT = np.ascontiguousarray(inp["hidden"][rows].T)
        fcw = inp["fc_W"][c * VC:(c + 1) * VC]
        fcwT = np.zeros((F, VP), dtype=BF)
        fcwT[:, :VC] = np.ascontiguousarray(fcw.T).astype(BF)
        fcb = np.full((1, VP), -30000.0, dtype=BF)
        fcb[0, :VC] = inp["fc_b"][c * VC:(c + 1) * VC].astype(BF)
        idxt = np.full((NPIECE, R, 128, 512), -1, dtype=np.int16)
        for (b, s, vl, r) in entry[c]:
            pc, off = vl // PW, vl % PW
            idxt[pc, r, b, 2 * s] = 2 * off
            idxt[pc, r, b, 2 * s + 1] = 2 * off + 1
        m = {
            "encT": encT,
            "encB": encB,
            "hTf": hT.astype(np.float32),
            "hTb": hT.astype(BF),
            "embT": np.ascontiguousarray(emb_all[rows].T).astype(BF),
            "biasJ": bias_j_all[rows],
            "biasC": bias_c_all[rows],
            "fcWT": fcwT,
            "fcb": fcb,
            "idxT": idxt.reshape(NPIECE * R * 128, 512),
        }
        m.update(shared)
        in_maps.append(m)
    return in_maps, R


_NC_CACHE = {}


def kernel(**inputs):
    in_maps, R = _prep(inputs)
    if R not in _NC_CACHE:
        _NC_CACHE[R] = build_nc(R)
    nc = _NC_CACHE[R]
    res = bass_utils.run_bass_kernel_spmd(nc, in_maps,
                                          core_ids=list(range(NCORES)))
    final = np.concatenate([res.results[c]["out_v"][:, :VC]
                            for c in range(NCORES)], axis=1)[:, :V]
    h_new = np.concatenate([res.results[c]["out_h"]
                            for c in range(NCORES)], axis=0)
    return final.astype(np.float32), h_new.astype(np.float32)


# revision 22
# speedup vs baseline: 1.1218x; 1.1218x over previous
"""Trainium2 Bass kernel for nn_DualGRUDecoder (scatter_memory).

Sharding: stage 1 data-parallel over batch (128 -> 16 rows/core) for
attention + GRU + copy-attention; all-gather of the GRU/attention state;
stage 2 vocab-parallel (50000 -> 6250 cols/core) for the fc projection,
softmax (cross-core stats exchange), and the scatter_add of copy
probabilities (done on-chip via gpsimd local_scatter with f32-as-u16
pairs). Host assembles: concat vocab shards (axis=1) + batch shards of
h_new (axis=0).
"""

import numpy as np
import ml_dtypes

import contextlib

import concourse.bacc as bacc
import concourse.bass as bass
import concourse.mybir as mybir
import concourse.tile as tile
from concourse import bass_utils

F32 = mybir.dt.float32
BF16 = mybir.dt.bfloat16
I16 = mybir.dt.int16
U16 = mybir.dt.uint16
BF = ml_dtypes.bfloat16

NCORES = 8
B, BC = 128, 16
S1, S2 = 200, 200
S = S1 + S2                  # 400
EMB, DEC, E2 = 256, 512, 1024
F = DEC + E2                 # 1536 state dim
XF = EMB + E2                # 1280 GRU input dim
V, VC, VP = 50000, 6250, 6272
PW, NPIECE = 896, 7          # scatter piece width (f32), 7*896 = 6272
NEG = -1e10

JROWS = S * BC               # 6400 joint-attention rows
CROWS = S1 * BC              # 3200 copy-attention rows
NTW = 400                    # energy N-tile width (25 s-values)
HALF = 3200                  # score half size (200 s-values)


DBG = False
NO_CC = False
NO_SCAT = False
NO_ENERGY = False
NO_FC = False
SMALL_OUT = False
STAGE1_ONLY = False


def build_nc(R):
    nc = bacc.Bacc("TRN2", target_bir_lowering=False, debug=False)
    nc.num_devices = NCORES

    def din(name, shape, dt):
        return nc.dram_tensor(name, shape, dt, kind="ExternalInput")

    # per-core activations
    enc_t_d = din("encT", [E2, JROWS], BF16)
    enc_b_d = din("encB", [8, S, 128, 16], BF16)
    ht_f_d = din("hTf", [DEC, BC], F32)
    ht_b_d = din("hTb", [DEC, BC], BF16)
    emb_t_d = din("embT", [EMB, BC], BF16)
    biasj_d = din("biasJ", [BC, S], F32)
    biasc_d = din("biasC", [BC, S1], F32)
    # replicated weights
    awe_d = din("aWeT", [E2, DEC], BF16)
    awh_d = din("aWhT", [DEC, DEC], BF16)
    ab_d = din("ab", [DEC, 1], F32)
    av_d = din("av", [DEC, 1], BF16)
    cwe_d = din("cWeT", [E2, DEC], BF16)
    cwh_d = din("cWhT", [DEC, DEC], BF16)
    cb_d = din("cb", [DEC, 1], F32)
    cv_d = din("cv", [DEC, 1], BF16)
    wih_d = din("WihT", [XF, 3 * DEC], BF16)
    whh_d = din("WhhT", [DEC, 3 * DEC], BF16)
    brz_d = din("brz", [2 * DEC, 1], F32)
    bhn_d = din("bhn", [DEC, 1], F32)
    bin_d = din("bin", [DEC, 1], F32)
    gw_d = din("gw", [XF, 1], BF16)
    gb_d = din("gb", [1, 1], F32)
    # vocab shard
    fcw_d = din("fcWT", [F, VP], BF16)
    fcb_d = din("fcb", [1, VP], BF16)
    idx_d = din("idxT", [NPIECE * R * 128, 512], I16)

    out_v = nc.dram_tensor("out_v", [B, 64 if SMALL_OUT else VP], F32, kind="ExternalOutput")
    out_h = nc.dram_tensor("out_h", [BC, DEC], F32, kind="ExternalOutput")
    if DBG:
        dbg_sc = nc.dram_tensor("dbg_sc", [16, S], F32, kind="ExternalOutput")
        dbg_a = nc.dram_tensor("dbg_a", [16, S], F32, kind="ExternalOutput")
        dbg_wt = nc.dram_tensor("dbg_wt", [128, 128], F32, kind="ExternalOutput")
        dbg_hn = nc.dram_tensor("dbg_hn", [128, 64], F32, kind="ExternalOutput")
        dbg_st2 = nc.dram_tensor("dbg_st2", [128, 1536], F32, kind="ExternalOutput")
        dbg_lg = nc.dram_tensor("dbg_lg", [128, 512], F32, kind="ExternalOutput")
        dbg_adf = nc.dram_tensor("dbg_adf", [128, 200], F32, kind="ExternalOutput")
        dbg_al = nc.dram_tensor("dbg_al", [128, 4], F32, kind="ExternalOutput")

    ag1_in = nc.dram_tensor("ag1_in", [F * BC], BF16)
    ag1_out = nc.dram_tensor("ag1_out", [NCORES * F * BC], BF16,
                             addr_space="Shared")
    ag2_in = nc.dram_tensor("ag2_in", [BC * 201], F32)
    ag2_out = nc.dram_tensor("ag2_out", [NCORES * BC * 201], F32,
                             addr_space="Shared")
    ag3_in = nc.dram_tensor("ag3_in", [B * 2], F32)
    ag3_out = nc.dram_tensor("ag3_out", [NCORES * B * 2], F32,
                             addr_space="Shared")
    scr_h = [nc.dram_tensor(f"scr_h{i}", [HALF], F32) for i in range(3)]
    scr_pg = nc.dram_tensor("scr_pg", [16], F32)
    scr_w = nc.dram_tensor("scr_w", [E2 * 16], F32)
    rg = [list(range(NCORES))]

    with tile.TileContext(nc) as tc, \
         tc.tile_pool(name="wp", bufs=1) as wp, \
         tc.tile_pool(name="s1", bufs=1) as s1, \
         tc.tile_pool(name="s2", bufs=3) as s2, \
         tc.tile_pool(name="sco", bufs=1) as sco, \
         tc.tile_pool(name="pmm", bufs=4, space="PSUM") as pmm, \
         tc.tile_pool(name="psm", bufs=2, space="PSUM") as psm, \
         tc.tile_pool(name="pw", bufs=1, space="PSUM") as pw:

        def energy_scores(we_sb, htb_sb, vv_sb, nhalves, tw):
            """Scores with b-major encT: one batch row per tile of width tw.
            htb_sb [128, 4, 16] = (W_h^T h + bias) per (d, b), fused into the
            tanh bias. Returns sc [16, 400] f32 (b, s)."""
            tph = HALF // tw          # tiles (= batch rows) per half
            sc = s1.tile([16, S], F32, tag="sc")
            for h in range(nhalves):
                half = sco.tile([1, HALF], F32, tag="scores")
                for ti in range(tph):
                    bg = h * tph + ti
                    n0 = ti * tw
                    et = s2.tile([128, 12, 512], BF16, tag="big_t")
                    nc.sync.dma_start(
                        et[:, 0:8, 0:tw],
                        enc_t_d[:, bg * 400: bg * 400 + tw].rearrange(
                            "(k p) n -> p k n", p=128))
                    th = s2.tile([128, 4, 400], BF16, tag="th_t")
                    for mc in range(4):
                        e_ps = pmm.tile([128, 512], F32, tag="mm")
                        for k in range(8):
                            nc.tensor.matmul(
                                e_ps[:, :tw],
                                we_sb[:, k, mc * 128:(mc + 1) * 128],
                                et[:, k, :tw],
                                start=(k == 0), stop=(k == 7))
                        nc.scalar.activation(
                            th[:, mc, :tw], e_ps[:, :tw],
                            mybir.ActivationFunctionType.Tanh,
                            bias=htb_sb[:, mc, bg:bg + 1])
                    sc_ps = psm.tile([1, 400], F32, tag="sm")
                    for mc in range(4):
                        nc.tensor.matmul(sc_ps[:, :tw], vv_sb[:, mc, :],
                                         th[:, mc, :tw],
                                         start=(mc == 0), stop=(mc == 3))
                    nc.scalar.copy(half[0:1, n0:n0 + tw], sc_ps[:, :tw])
                hd = scr_h[h if nhalves == 2 else 2]
                nc.sync.dma_start(hd[:].rearrange("(o n) -> o n", o=1), half[:])
                nc.sync.dma_start(
                    sc[h * tph:(h + 1) * tph, :tw],
                    hd[:].rearrange("(b s) -> b s", b=tph))
            return sc

        def softmax_rows(sc, ns, bias_d):
            bb = s1.tile([16, S], F32, tag="bias")
            nc.sync.dma_start(bb[:, :ns], bias_d[:])
            nc.vector.tensor_add(sc[:, :ns], sc[:, :ns], bb[:, :ns])
            mx = s1.tile([16, 1], F32, tag="mx")
            nc.vector.reduce_max(mx[:], sc[:, :ns], axis=mybir.AxisListType.X)
            nmx = s1.tile([16, 1], F32, tag="nmx")
            nc.vector.tensor_scalar_mul(nmx[:], mx[:], -1.0)
            ex = s1.tile([16, S], F32, tag="ex")
            nc.scalar.activation(ex[:, :ns], sc[:, :ns],
                                 mybir.ActivationFunctionType.Exp,
                                 bias=nmx[:])
            sm = s1.tile([16, 1], F32, tag="sm1")
            nc.vector.reduce_sum(sm[:], ex[:, :ns], axis=mybir.AxisListType.X)
            si = s1.tile([16, 1], F32, tag="si")
            nc.vector.reciprocal(si[:], sm[:])
            a = s1.tile([16, S], F32, tag="a")
            nc.vector.tensor_scalar_mul(a[:, :ns], ex[:, :ns], si[:])
            return a

        # ---------- persistent weight loads ----------
        awe = wp.tile([128, 8, 512], BF16, tag="we")
        nc.sync.dma_start(awe[:], awe_d[:].rearrange("(k p) m -> p k m", p=128))
        awh = wp.tile([128, 4, 512], BF16, tag="wh")
        nc.sync.dma_start(awh[:], awh_d[:].rearrange("(k p) m -> p k m", p=128))
        vj = wp.tile([128, 4, 1], BF16, tag="vj")
        nc.sync.dma_start(vj[:], av_d[:].rearrange("(k p) o -> p k o", p=128))
        vc = wp.tile([128, 4, 1], BF16, tag="vc")
        nc.sync.dma_start(vc[:], cv_d[:].rearrange("(k p) o -> p k o", p=128))
        ab = wp.tile([128, 4, 1], F32, tag="ab")
        nc.sync.dma_start(ab[:], ab_d[:].rearrange("(k p) o -> p k o", p=128))
        cb = wp.tile([128, 4, 1], F32, tag="cb")
        nc.sync.dma_start(cb[:], cb_d[:].rearrange("(k p) o -> p k o", p=128))
        brz = wp.tile([128, 8, 1], F32, tag="brz")
        nc.sync.dma_start(brz[:], brz_d[:].rearrange("(k p) o -> p k o", p=128))
        bhn = wp.tile([128, 4, 1], F32, tag="bhn")
        nc.sync.dma_start(bhn[:], bhn_d[:].rearrange("(k p) o -> p k o", p=128))
        bin_ = wp.tile([128, 4, 1], F32, tag="bin")
        nc.sync.dma_start(bin_[:], bin_d[:].rearrange("(k p) o -> p k o", p=128))
        gw = wp.tile([128, 10, 1], BF16, tag="gw")
        nc.sync.dma_start(gw[:], gw_d[:].rearrange("(k p) o -> p k o", p=128))
        gb = wp.tile([1, 1], F32, tag="gb")
        nc.sync.dma_start(gb[:], gb_d[:])
        hTb = wp.tile([128, 4, 16], BF16, tag="hTb")
        nc.sync.dma_start(hTb[:], ht_b_d[:].rearrange("(k p) b -> p k b", p=128))
        hTf = wp.tile([128, 4, 16], F32, tag="hTf")
        nc.sync.dma_start(hTf[:], ht_f_d[:].rearrange("(k p) b -> p k b", p=128))
        ones1 = wp.tile([1, 128], BF16, tag="ones1")
        nc.vector.memset(ones1[:], 1.0)
        onesP = wp.tile([128, 1], BF16, tag="onesP")
        nc.vector.memset(onesP[:], 1.0)

        # xt = GRU input x^T [128, 10, 16]: chunks 0:2 emb, 2:10 weighted
        xt = wp.tile([128, 10, 16], BF16, tag="xt")
        nc.sync.dma_start(xt[:, 0:2, :],
                          emb_t_d[:].rearrange("(k p) b -> p k b", p=128))
        # st = state^T payload [128, 12, 16]: 0:4 h_new, 4:12 weighted
        st = wp.tile([128, 12, 16], BF16, tag="st")
        logits = wp.tile([128, VP], F32, tag="logits")

        with tc.tile_pool(name="gp", bufs=1) as gp:
            wih = gp.tile([128, 10, 1536], BF16, tag="wih")
            nc.sync.dma_start(wih[:],
                              wih_d[:].rearrange("(k p) m -> p k m", p=128))
            whh = gp.tile([128, 4, 1536], BF16, tag="whh")
            nc.sync.dma_start(whh[:],
                              whh_d[:].rearrange("(k p) m -> p k m", p=128))

            # ---------- joint attention ----------
            htj = s1.tile([128, 4, 16], F32, tag="htj")
            for mc in range(4):
                hp = psm.tile([128, 16], F32, tag="sm")
                for k in range(4):
                    nc.tensor.matmul(
                        hp[:], awh[:, k, mc * 128:(mc + 1) * 128],
                        hTb[:, k, :], start=(k == 0), stop=(k == 3))
                nc.scalar.activation(htj[:, mc, :], hp[:],
                                     mybir.ActivationFunctionType.Identity,
                                     bias=ab[:, mc, :])

            sc_j = energy_scores(awe, htj, vj, 2, 400)
            if DBG:
                nc.sync.dma_start(dbg_sc[:], sc_j[:])
            a_j = softmax_rows(sc_j, S, biasj_d)
            if DBG:
                nc.sync.dma_start(dbg_a[:], a_j[:])

            # transpose a_j -> aT [128, 4, 32] bf16 (s on partitions)
            a32 = s1.tile([32, 416], BF16, tag="a32")
            nc.vector.memset(a32[:], 0.0)
            nc.vector.tensor_copy(a32[:16, :S], a_j[:, :S])
            aT = s1.tile([128, 4, 32], BF16, tag="aT")
            for k in range(4):
                nj = 4 if k < 3 else 1
                for j32 in range(nj):
                    nc.vector.transpose(
                        aT[j32 * 32:(j32 + 1) * 32, k, 0:32],
                        a32[0:32, k * 128 + j32 * 32: k * 128 + (j32 + 1) * 32])

            # ---------- weighted sum: wT[e, b] = sum_s a[b,s] enc[s,b,e] ----------
            wT = s1.tile([128, 8, 16], F32, tag="wT")
            wd = scr_w
            for ec in range(16):
                c8, half64 = ec // 2, (ec % 2) * 64
                wps = pw.tile([1, 1024], F32, tag="wps")
                for k in range(4):
                    sl = 128 if k < 3 else 16
                    nB = s2.tile([128, 1024], BF16, tag="nB")
                    nc.sync.dma_start(
                        nB[:sl, :],
                        enc_b_d[c8, k * 128: k * 128 + sl,
                                half64:half64 + 64].rearrange(
                                    "s e j -> s (e j)"))
                    tmp = s2.tile([128, 1024], BF16, tag="wtmp")
                    nc.vector.tensor_mul(
                        tmp[:sl, :].rearrange("p (e j) -> p e j", j=16),
                        nB[:sl, :].rearrange("p (e j) -> p e j", j=16),
                        aT[:sl, k, None, 0:16].broadcast_to([sl, 64, 16]))
                    for q in range(2):
                        nc.tensor.matmul(
                            wps[0:1, q * 512:(q + 1) * 512],
                            onesP[:sl, :], tmp[:sl, q * 512:(q + 1) * 512],
                            start=(k == 0), stop=(k == 3))
                wst = s2.tile([1, 1024], F32, tag="wst")
                nc.scalar.copy(wst[:], wps[:])
                nc.sync.dma_start(
                    wd[ec * 1024:(ec + 1) * 1024].rearrange("(o n) -> o n", o=1),
                    wst[:])
            nc.sync.dma_start(
                wT[:], wd[:].rearrange("(c p j) -> p c j", p=128, j=16))
            if DBG:
                nc.sync.dma_start(dbg_wt[:], wT[:].rearrange("p c j -> p (c j)"))
            nc.vector.tensor_copy(xt[:, 2:10, :], wT[:])
            nc.vector.tensor_copy(st[:, 4:12, :], wT[:])

            # ---------- p_gen ----------
            pgp = psm.tile([1, 16], F32, tag="sm")
            for k in range(10):
                nc.tensor.matmul(pgp[:], gw[:, k, :], xt[:, k, :],
                                 start=(k == 0), stop=(k == 9))
            pgT = s1.tile([1, 16], F32, tag="pgT")
            nc.scalar.activation(pgT[:], pgp[:],
                                 mybir.ActivationFunctionType.Sigmoid,
                                 bias=gb[:])
            pgd = scr_pg
            nc.sync.dma_start(pgd[:].rearrange("(o n) -> o n", o=1), pgT[:])
            pg_b = s1.tile([16, 1], F32, tag="pg_b")
            nc.sync.dma_start(pg_b[:], pgd[:].rearrange("(b o) -> b o", o=1))

            # ---------- GRU ----------
            rz = s1.tile([128, 8, 16], F32, tag="rz")
            hnewT = s1.tile([128, 4, 16], F32, tag="hnewT")
            for mc in range(8):
                gpsum = psm.tile([128, 16], F32, tag="sm")
                for k in range(10):
                    nc.tensor.matmul(
                        gpsum[:], wih[:, k, mc * 128:(mc + 1) * 128],
                        xt[:, k, :], start=(k == 0), stop=False)
                for k in range(4):
                    nc.tensor.matmul(
                        gpsum[:], whh[:, k, mc * 128:(mc + 1) * 128],
                        hTb[:, k, :], start=False, stop=(k == 3))
                nc.scalar.activation(rz[:, mc, :], gpsum[:],
                                     mybir.ActivationFunctionType.Sigmoid,
                                     bias=brz[:, mc, :])
            for mc in range(4):
                m8 = mc + 8
                gx = psm.tile([128, 16], F32, tag="sm")
                for k in range(10):
                    nc.tensor.matmul(
                        gx[:], wih[:, k, m8 * 128:(m8 + 1) * 128],
                        xt[:, k, :], start=(k == 0), stop=(k == 9))
                gh = psm.tile([128, 16], F32, tag="sm")
                for k in range(4):
                    nc.tensor.matmul(
                        gh[:], whh[:, k, m8 * 128:(m8 + 1) * 128],
                        hTb[:, k, :], start=(k == 0), stop=(k == 3))
                hnb = s1.tile([128, 16], F32, tag="hnb")
                nc.scalar.activation(hnb[:], gh[:],
                                     mybir.ActivationFunctionType.Identity,
                                     bias=bhn[:, mc, :])
                rhn = s1.tile([128, 16], F32, tag="rhn")
                nc.vector.tensor_mul(rhn[:], rz[:, mc, :], hnb[:])
                xnr = s1.tile([128, 16], F32, tag="xnr")
                nc.vector.tensor_add(xnr[:], gx[:], rhn[:])
                n_sb = s1.tile([128, 16], F32, tag="n_sb")
                nc.scalar.activation(n_sb[:], xnr[:],
                                     mybir.ActivationFunctionType.Tanh,
                                     bias=bin_[:, mc, :])
                dh = s1.tile([128, 16], F32, tag="dh")
                nc.vector.tensor_sub(dh[:], hTf[:, mc, :], n_sb[:])
                zd = s1.tile([128, 16], F32, tag="zd")
                nc.vector.tensor_mul(zd[:], rz[:, mc + 4, :], dh[:])
                nc.vector.tensor_add(hnewT[:, mc, :], n_sb[:], zd[:])
                nc.vector.tensor_copy(st[:, mc, :], hnewT[:, mc, :])

        if DBG:
            nc.sync.dma_start(dbg_hn[:], hnewT[:].rearrange("p c j -> p (c j)"))
        # h_new natural layout -> out_h
        hp32 = s1.tile([128, 32], F32, tag="hp32")
        nc.vector.memset(hp32[:], 0.0)
        hnat = s1.tile([32, DEC], F32, tag="hnat")
        for mc in range(4):
            nc.vector.tensor_copy(hp32[:, 0:16], hnewT[:, mc, :])
            for pb in range(4):
                nc.vector.transpose(
                    hnat[0:32, mc * 128 + pb * 32: mc * 128 + (pb + 1) * 32],
                    hp32[pb * 32:(pb + 1) * 32, 0:32])
        nc.sync.dma_start(out_h[:], hnat[:16, :])

        # ---------- AG1: state ----------
        nc.sync.dma_start(
            ag1_in[:].rearrange("(p c j) -> p (c j)", p=128, j=16), st[:])
        if NO_CC:
            nc.sync.dma_start(ag1_out[0:F * BC], ag1_in[:])
        else:
            nc.gpsimd.collective_compute(
                "AllGather", mybir.AluOpType.bypass, replica_groups=rg,
                ins=[ag1_in[:]], outs=[ag1_out[:]])

        # stage-2 pools: reuse the SBUF freed by the GRU-weight pool
        es = contextlib.ExitStack()
        s3 = es.enter_context(tc.tile_pool(name="s3", bufs=1))
        s4 = es.enter_context(tc.tile_pool(name="s4", bufs=2))
        delta = s3.tile([128, VP], F32, tag="delta")

        # ---------- copy attention ----------
        cwe = wp.tile([128, 8, 512], BF16, tag="we")
        nc.sync.dma_start(cwe[:], cwe_d[:].rearrange("(k p) m -> p k m", p=128))
        cwh = wp.tile([128, 4, 512], BF16, tag="wh")
        nc.sync.dma_start(cwh[:], cwh_d[:].rearrange("(k p) m -> p k m", p=128))
        htc = s1.tile([128, 4, 16], F32, tag="htc")
        for mc in range(4):
            hp = psm.tile([128, 16], F32, tag="sm")
            for k in range(4):
                nc.tensor.matmul(
                    hp[:], cwh[:, k, mc * 128:(mc + 1) * 128],
                    st[:, k, :], start=(k == 0), stop=(k == 3))
            nc.scalar.activation(htc[:, mc, :], hp[:],
                                 mybir.ActivationFunctionType.Identity,
                                 bias=cb[:, mc, :])
        sc_c = energy_scores(cwe, htc, vc, 1, 200)
        a_c = softmax_rows(sc_c, S1, biasc_d)
        adp = s1.tile([16, 201], F32, tag="adp")
        apg = s1.tile([16, S1], F32, tag="apg")
        nc.vector.tensor_scalar_mul(apg[:], a_c[:, :S1], pg_b[:])
        nc.vector.tensor_sub(adp[:, 0:S1], a_c[:, :S1], apg[:])
        nc.vector.tensor_copy(adp[:, 200:201], pg_b[:])
        nc.sync.dma_start(
            ag2_in[:].rearrange("(b c) -> b c", b=16), adp[:])
        if NO_CC:
            nc.sync.dma_start(ag2_out[0:BC * 201], ag2_in[:])
        else:
            nc.gpsimd.collective_compute(
                "AllGather", mybir.AluOpType.bypass, replica_groups=rg,
                ins=[ag2_in[:]], outs=[ag2_out[:]])

        # ---------- stage 2: fc matmul over vocab shard ----------
        st2 = s1.tile([128, 12, 128], BF16, tag="st2")
        nc.sync.dma_start(
            st2[:], ag1_out[:].rearrange("(r p c j) -> p c r j",
                                         r=NCORES, p=128, j=16))
        if DBG:
            st2f = s1.tile([128, 12, 128], F32, tag="st2f")
            nc.vector.tensor_copy(st2f[:], st2[:])
            nc.sync.dma_start(dbg_st2[:], st2f[:].rearrange("p c j -> p (c j)"))
        if NO_FC:
            nc.vector.memset(logits[:], 0.001)
        v0 = 0
        while v0 < (VP if not NO_FC else 0):
            nv = min(512, VP - v0)
            fw = s4.tile([128, 12, 512], BF16, tag="fw")
            nc.sync.dma_start(
                fw[:, :, :nv],
                fcw_d[:, v0:v0 + nv].rearrange("(k p) n -> p k n", p=128))
            fcbt = s2.tile([1, 512], BF16, tag="fcb_t")
            nc.sync.dma_start(fcbt[0:1, :nv], fcb_d[0:1, v0:v0 + nv])
            lp = pmm.tile([128, 512], F32, tag="mm")
            for k in range(12):
                nc.tensor.matmul(lp[:, :nv], st2[:, k, :], fw[:, k, :nv],
                                 start=(k == 0), stop=False)
            nc.tensor.matmul(lp[:, :nv], ones1[:], fcbt[0:1, :nv],
                             start=False, stop=True)
            nc.scalar.copy(logits[:, v0:v0 + nv], lp[:, :nv])
            v0 += nv

        if DBG:
            nc.sync.dma_start(dbg_lg[:], logits[:, 0:512])
        m_c = s1.tile([128, 1], F32, tag="m_c")
        nc.vector.reduce_max(m_c[:], logits[:], axis=mybir.AxisListType.X)
        nmc = s1.tile([128, 1], F32, tag="nmc")
        nc.vector.tensor_scalar_mul(nmc[:], m_c[:], -1.0)
        nc.scalar.activation(logits[:], logits[:],
                             mybir.ActivationFunctionType.Exp, bias=nmc[:])
        s_c = s1.tile([128, 1], F32, tag="s_c")
        nc.vector.reduce_sum(s_c[:], logits[:], axis=mybir.AxisListType.X)

        stats = s1.tile([128, 2], F32, tag="stats")
        nc.vector.tensor_copy(stats[:, 0:1], m_c[:])
        nc.vector.tensor_copy(stats[:, 1:2], s_c[:])
        nc.sync.dma_start(
            ag3_in[:].rearrange("(p c) -> p c", p=128), stats[:])
        if NO_CC:
            nc.sync.dma_start(ag3_out[0:B * 2], ag3_in[:])
        else:
            nc.gpsimd.collective_compute(
                "AllGather", mybir.AluOpType.bypass, replica_groups=rg,
                ins=[ag3_in[:]], outs=[ag3_out[:]])

        mg = s1.tile([128, 8, 2], F32, tag="mg")
        nc.sync.dma_start(
            mg[:], ag3_out[:].rearrange("(r p c) -> p r c", r=NCORES, p=128))
        M = s1.tile([128, 1], F32, tag="M")
        nc.vector.reduce_max(M[:], mg[:, :, 0:1], axis=mybir.AxisListType.XY)
        nM = s1.tile([128, 1], F32, tag="nM")
        nc.vector.tensor_scalar_mul(nM[:], M[:], -1.0)
        eR = s1.tile([128, 8], F32, tag="eR")
        nc.scalar.activation(eR[:], mg[:, :, 0],
                             mybir.ActivationFunctionType.Exp, bias=nM[:])
        t8 = s1.tile([128, 8], F32, tag="t8")
        nc.vector.tensor_mul(t8[:], eR[:], mg[:, :, 1])
        Ssum = s1.tile([128, 1], F32, tag="Ssum")
        nc.vector.reduce_sum(Ssum[:], t8[:], axis=mybir.AxisListType.X)
        Sinv = s1.tile([128, 1], F32, tag="Sinv")
        nc.vector.reciprocal(Sinv[:], Ssum[:])
        emd = s1.tile([128, 1], F32, tag="emd")
        nc.scalar.activation(emd[:], m_c[:],
                             mybir.ActivationFunctionType.Exp, bias=nM[:])
        pgf = s1.tile([128, 1], F32, tag="pgf")
        nc.sync.dma_start(
            pgf[:], ag2_out[:].rearrange("(b c) -> b c", c=201)[:, 200:201])
        alpha = s1.tile([128, 1], F32, tag="alpha")
        nc.vector.tensor_mul(alpha[:], emd[:], Sinv[:])
        nc.vector.tensor_mul(alpha[:], alpha[:], pgf[:])

        # ---------- scatter_add of copy probabilities ----------
        adf = s1.tile([128, 200], F32, tag="adf")
        nc.sync.dma_start(
            adf[:], ag2_out[:].rearrange("(b c) -> b c", c=201)[:, 0:200])
        if DBG:
            nc.sync.dma_start(dbg_adf[:], adf[:])
            dal = s1.tile([128, 4], F32, tag="dal")
            nc.vector.tensor_copy(dal[:, 0:1], alpha[:])
            nc.vector.tensor_copy(dal[:, 1:2], m_c[:])
            nc.vector.tensor_copy(dal[:, 2:3], s_c[:])
            nc.vector.tensor_copy(dal[:, 3:4], Ssum[:])
            nc.sync.dma_start(dbg_al[:], dal[:])
        for pc in range(NPIECE if not NO_SCAT else 0):
            for r in range(R):
                ix = s2.tile([128, 512], I16, tag="idx_t")
                nc.sync.dma_start(
                    ix[:],
                    idx_d[(pc * R + r) * 128:(pc * R + r + 1) * 128, :])
                dl = s2.tile([128, PW], F32, tag="dl")
                nc.gpsimd.local_scatter(
                    dl[:].bitcast(U16), adf[:].bitcast(U16), ix[:, 0:400],
                    channels=128, num_elems=2 * PW, num_idxs=400)
                if r == 0:
                    nc.vector.tensor_copy(delta[:, pc * PW:(pc + 1) * PW], dl[:])
                else:
                    nc.vector.tensor_add(delta[:, pc * PW:(pc + 1) * PW],
                                         delta[:, pc * PW:(pc + 1) * PW],
                                         dl[:])
        nc.vector.scalar_tensor_tensor(
            out=logits[:], in0=logits[:], scalar=alpha[:], in1=delta[:],
            op0=mybir.AluOpType.mult, op1=mybir.AluOpType.add)
        nc.sync.dma_start(out_v[:], logits[:, 0:64] if SMALL_OUT else logits[:])
        es.close()

    nc.compile()
    return nc


def _prep(inputs):
    """Host-side sharding/layout prep. Returns (in_maps, R)."""
    inp = {k: np.asarray(v) for k, v in inputs.items()}
    e1, e2 = inp["encoder_outputs1"], inp["encoder_outputs2"]
    src1 = inp["src1"].astype(np.int64)
    emb_all = inp["embedding"][inp["input"].astype(np.int64)]  # [B, EMB]
    attn_W, attn_b, attn_v = inp["attn_W"], inp["attn_b"], inp["attn_v"]
    copy_W, copy_b, copy_v = inp["copy_W"], inp["copy_b"], inp["copy_v"]
    mask_j = np.concatenate([inp["mask1"], inp["mask2"]], axis=1)
    mask_c = inp["mask1"] * inp["triple_mask"]
    bias_j_all = np.where(mask_j == 0, NEG, 0.0).astype(np.float32)
    bias_c_all = np.where(mask_c == 0, NEG, 0.0).astype(np.float32)

    shared = {
        "aWeT": attn_W[DEC:].astype(BF), "aWhT": np.ascontiguousarray(attn_W[:DEC]).astype(BF),
        "ab": attn_b[:, None].astype(np.float32), "av": attn_v[:, None].astype(BF),
        "cWeT": copy_W[DEC:].astype(BF), "cWhT": np.ascontiguousarray(copy_W[:DEC]).astype(BF),
        "cb": copy_b[:, None].astype(np.float32), "cv": copy_v[:, None].astype(BF),
        "WihT": np.ascontiguousarray(inp["gru_Wih"].T).astype(BF),
        "WhhT": np.ascontiguousarray(inp["gru_Whh"].T).astype(BF),
        "brz": (inp["gru_bih"] + inp["gru_bhh"])[:2 * DEC, None].astype(np.float32),
        "bhn": inp["gru_bhh"][2 * DEC:, None].astype(np.float32),
        "bin": inp["gru_bih"][2 * DEC:, None].astype(np.float32),
        "gw": np.ascontiguousarray(
            np.concatenate([inp["gate_W"][0, E2:], inp["gate_W"][0, :E2]])[:, None]).astype(BF),
        "gb": inp["gate_b"].reshape(1, 1).astype(np.float32),
    }

    # scatter plan: occurrence rounds
    occ = {}
    rmax = 1
    entry = [[] for _ in range(NCORES)]
    for s in range(S1):
        for b in range(B):
            v = int(src1[s, b])
            c = min(v // VC, NCORES - 1)
            vl = v - c * VC
            key = (b, v)
            r = occ.get(key, 0)
            occ[key] = r + 1
            rmax = max(rmax, r + 1)
            entry[c].append((b, s, vl, r))
    R = rmax

    in_maps = []
    for c in range(NCORES):
        rows = slice(c * BC, (c + 1) * BC)
        enc = np.concatenate([e1[:, rows], e2[:, rows]], axis=0).astype(BF)
        encT = np.ascontiguousarray(enc.transpose(2, 1, 0)).reshape(E2, JROWS)
        encB = np.ascontiguousarray(
            enc.reshape(S, BC, 8, 128).transpose(2, 0, 3, 1))
        hT = np.ascontiguousarray(inp["hidden"][rows].T)
        fcw = inp["fc_W"][c * VC:(c + 1) * VC]
        fcwT = np.zeros((F, VP), dtype=BF)
        fcwT[:, :VC] = np.ascontiguousarray(fcw.T).astype(BF)
        fcb = np.full((1, VP), -30000.0, dtype=BF)
        fcb[0, :VC] = inp["fc_b"][c * VC:(c + 1) * VC].astype(BF)
        idxt = np.full((NPIECE, R, 128, 512), -1, dtype=np.int16)
        for (b, s, vl, r) in entry[c]:
            pc, off = vl // PW, vl % PW
            idxt[pc, r, b, 2 * s] = 2 * off
            idxt[pc, r, b, 2 * s + 1] = 2 * off + 1
        m = {
            "encT": encT,
            "encB": encB,
            "hTf": hT.astype(np.float32),
            "hTb": hT.astype(BF),
            "embT": np.ascontiguousarray(emb_all[rows].T).astype(BF),
            "biasJ": bias_j_all[rows],
            "biasC": bias_c_all[rows],
            "fcWT": fcwT,
            "fcb": fcb,
            "idxT": idxt.reshape(NPIECE * R * 128, 512),
        }
        m.update(shared)
        in_maps.append(m)
    return in_maps, R


_NC_CACHE = {}


def kernel(**inputs):
    in_maps, R = _prep(inputs)
    if R not in _NC_CACHE:
        _NC_CACHE[R] = build_nc(R)
    nc = _NC_CACHE[R]
    res = bass_utils.run_bass_kernel_spmd(nc, in_maps,
                                          core_ids=list(range(NCORES)))
    final = np.concatenate([res.results[c]["out_v"][:, :VC]
                            for c in range(NCORES)], axis=1)[:, :V]
    h_new = np.concatenate([res.results[c]["out_h"]
                            for c in range(NCORES)], axis=0)
    return final.astype(np.float32), h_new.astype(np.float32)


# revision 24
# speedup vs baseline: 1.2071x; 1.0761x over previous
"""Trainium2 Bass kernel for nn_DualGRUDecoder (scatter_memory).

Sharding: stage 1 data-parallel over batch (128 -> 16 rows/core) for
attention + GRU + copy-attention; all-gather of the GRU/attention state;
stage 2 vocab-parallel (50000 -> 6250 cols/core) for the fc projection,
softmax (cross-core stats exchange), and the scatter_add of copy
probabilities (done on-chip via gpsimd local_scatter with f32-as-u16
pairs). Host assembles: concat vocab shards (axis=1) + batch shards of
h_new (axis=0).
"""

import numpy as np
import ml_dtypes

import contextlib

import concourse.bacc as bacc
import concourse.bass as bass
import concourse.mybir as mybir
import concourse.tile as tile
from concourse import bass_utils

F32 = mybir.dt.float32
BF16 = mybir.dt.bfloat16
I16 = mybir.dt.int16
U16 = mybir.dt.uint16
BF = ml_dtypes.bfloat16

NCORES = 8
B, BC = 128, 16
S1, S2 = 200, 200
S = S1 + S2                  # 400
EMB, DEC, E2 = 256, 512, 1024
F = DEC + E2                 # 1536 state dim
XF = EMB + E2                # 1280 GRU input dim
V, VC, VP = 50000, 6250, 6272
PW, NPIECE = 896, 7          # scatter piece width (f32), 7*896 = 6272
NEG = -1e10

JROWS = S * BC               # 6400 joint-attention rows
CROWS = S1 * BC              # 3200 copy-attention rows
NTW = 400                    # energy N-tile width (25 s-values)
HALF = 3200                  # score half size (200 s-values)


DBG = False
NO_CC = False
NO_SCAT = False
NO_ENERGY = False
NO_FC = False
SMALL_OUT = False
STAGE1_ONLY = False


def build_nc(R):
    nc = bacc.Bacc("TRN2", target_bir_lowering=False, debug=False)
    nc.num_devices = NCORES

    def din(name, shape, dt):
        return nc.dram_tensor(name, shape, dt, kind="ExternalInput")

    # per-core activations
    enc_t_d = din("encT", [E2, JROWS], BF16)
    enc_b_d = din("encB", [8, S, 128, 16], BF16)
    ht_f_d = din("hTf", [DEC, BC], F32)
    ht_b_d = din("hTb", [DEC, BC], BF16)
    emb_t_d = din("embT", [EMB, BC], BF16)
    biasj_d = din("biasJ", [BC, S], F32)
    biasc_d = din("biasC", [BC, S1], F32)
    # replicated weights
    awe_d = din("aWeT", [E2, DEC], BF16)
    awh_d = din("aWhT", [DEC, DEC], BF16)
    ab_d = din("ab", [DEC, 1], F32)
    av_d = din("av", [DEC, 1], BF16)
    cwe_d = din("cWeT", [E2, DEC], BF16)
    cwh_d = din("cWhT", [DEC, DEC], BF16)
    cb_d = din("cb", [DEC, 1], F32)
    cv_d = din("cv", [DEC, 1], BF16)
    wih_d = din("WihT", [XF, 3 * DEC], BF16)
    whh_d = din("WhhT", [DEC, 3 * DEC], BF16)
    brz_d = din("brz", [2 * DEC, 1], F32)
    bhn_d = din("bhn", [DEC, 1], F32)
    bin_d = din("bin", [DEC, 1], F32)
    gw_d = din("gw", [XF, 1], BF16)
    gb_d = din("gb", [1, 1], F32)
    # vocab shard
    fcw_d = din("fcWT", [F, VP], BF16)
    fcb_d = din("fcb", [1, VP], BF16)
    idx_d = din("idxT", [NPIECE * R * 128, 512], I16)

    out_v = nc.dram_tensor("out_v", [B, 64 if SMALL_OUT else VP], F32, kind="ExternalOutput")
    out_h = nc.dram_tensor("out_h", [BC, DEC], F32, kind="ExternalOutput")
    if DBG:
        dbg_sc = nc.dram_tensor("dbg_sc", [16, S], F32, kind="ExternalOutput")
        dbg_a = nc.dram_tensor("dbg_a", [16, S], F32, kind="ExternalOutput")
        dbg_wt = nc.dram_tensor("dbg_wt", [128, 128], F32, kind="ExternalOutput")
        dbg_hn = nc.dram_tensor("dbg_hn", [128, 64], F32, kind="ExternalOutput")
        dbg_st2 = nc.dram_tensor("dbg_st2", [128, 1536], F32, kind="ExternalOutput")
        dbg_lg = nc.dram_tensor("dbg_lg", [128, 512], F32, kind="ExternalOutput")
        dbg_adf = nc.dram_tensor("dbg_adf", [128, 200], F32, kind="ExternalOutput")
        dbg_al = nc.dram_tensor("dbg_al", [128, 4], F32, kind="ExternalOutput")

    ag1_in = nc.dram_tensor("ag1_in", [F * BC], BF16)
    ag1_out = nc.dram_tensor("ag1_out", [NCORES * F * BC], BF16,
                             addr_space="Shared")
    ag2_in = nc.dram_tensor("ag2_in", [BC * 201], F32)
    ag2_out = nc.dram_tensor("ag2_out", [NCORES * BC * 201], F32,
                             addr_space="Shared")
    ag3_in = nc.dram_tensor("ag3_in", [B * 2], F32)
    ag3_out = nc.dram_tensor("ag3_out", [NCORES * B * 2], F32,
                             addr_space="Shared")
    scr_h = [nc.dram_tensor(f"scr_h{i}", [HALF], F32) for i in range(3)]
    scr_pg = nc.dram_tensor("scr_pg", [16], F32)
    scr_w = nc.dram_tensor("scr_w", [E2 * 16], F32)
    rg = [list(range(NCORES))]

    with tile.TileContext(nc) as tc, \
         tc.tile_pool(name="wp", bufs=1) as wp, \
         tc.tile_pool(name="s1", bufs=1) as s1, \
         tc.tile_pool(name="s2", bufs=4) as s2, \
         tc.tile_pool(name="sco", bufs=1) as sco, \
         tc.tile_pool(name="pmm", bufs=4, space="PSUM") as pmm, \
         tc.tile_pool(name="psm", bufs=2, space="PSUM") as psm, \
         tc.tile_pool(name="pw", bufs=1, space="PSUM") as pw:

        def energy_scores(we_sb, htb_sb, vv_sb, nhalves, tw):
            """Scores with b-major encT: one batch row per tile of width tw.
            htb_sb [128, 4, 16] = (W_h^T h + bias) per (d, b), fused into the
            tanh bias. Returns sc [16, 400] f32 (b, s)."""
            tph = HALF // tw          # tiles (= batch rows) per half
            sc = s1.tile([16, S], F32, tag="sc")
            for h in range(nhalves):
                half = sco.tile([1, HALF], F32, tag="scores")
                for ti in range(tph):
                    bg = h * tph + ti
                    n0 = ti * tw
                    et = s2.tile([128, 8, 512], BF16, tag="big_t")
                    nc.sync.dma_start(
                        et[:, 0:8, 0:tw],
                        enc_t_d[:, bg * 400: bg * 400 + tw].rearrange(
                            "(k p) n -> p k n", p=128))
                    th = s2.tile([128, 4, 400], BF16, tag="th_t")
                    for mc in range(4):
                        e_ps = pmm.tile([128, 512], F32, tag="mm")
                        for k in range(8):
                            nc.tensor.matmul(
                                e_ps[:, :tw],
                                we_sb[:, k, mc * 128:(mc + 1) * 128],
                                et[:, k, :tw],
                                start=(k == 0), stop=(k == 7))
                        nc.scalar.activation(
                            th[:, mc, :tw], e_ps[:, :tw],
                            mybir.ActivationFunctionType.Tanh,
                            bias=htb_sb[:, mc, bg:bg + 1])
                    sc_ps = psm.tile([1, 400], F32, tag="sm")
                    for mc in range(4):
                        nc.tensor.matmul(sc_ps[:, :tw], vv_sb[:, mc, :],
                                         th[:, mc, :tw],
                                         start=(mc == 0), stop=(mc == 3))
                    nc.scalar.copy(half[0:1, n0:n0 + tw], sc_ps[:, :tw])
                hd = scr_h[h if nhalves == 2 else 2]
                nc.sync.dma_start(hd[:].rearrange("(o n) -> o n", o=1), half[:])
                nc.sync.dma_start(
                    sc[h * tph:(h + 1) * tph, :tw],
                    hd[:].rearrange("(b s) -> b s", b=tph))
            return sc

        def softmax_rows(sc, ns, bias_d):
            bb = s1.tile([16, S], F32, tag="bias")
            nc.sync.dma_start(bb[:, :ns], bias_d[:])
            nc.vector.tensor_add(sc[:, :ns], sc[:, :ns], bb[:, :ns])
            mx = s1.tile([16, 1], F32, tag="mx")
            nc.vector.reduce_max(mx[:], sc[:, :ns], axis=mybir.AxisListType.X)
            nmx = s1.tile([16, 1], F32, tag="nmx")
            nc.vector.tensor_scalar_mul(nmx[:], mx[:], -1.0)
            ex = s1.tile([16, S], F32, tag="ex")
            nc.scalar.activation(ex[:, :ns], sc[:, :ns],
                                 mybir.ActivationFunctionType.Exp,
                                 bias=nmx[:])
            sm = s1.tile([16, 1], F32, tag="sm1")
            nc.vector.reduce_sum(sm[:], ex[:, :ns], axis=mybir.AxisListType.X)
            si = s1.tile([16, 1], F32, tag="si")
            nc.vector.reciprocal(si[:], sm[:])
            a = s1.tile([16, S], F32, tag="a")
            nc.vector.tensor_scalar_mul(a[:, :ns], ex[:, :ns], si[:])
            return a

        # ---------- persistent weight loads ----------
        awe = wp.tile([128, 8, 512], BF16, tag="we")
        nc.sync.dma_start(awe[:], awe_d[:].rearrange("(k p) m -> p k m", p=128))
        awh = wp.tile([128, 4, 512], BF16, tag="wh")
        nc.sync.dma_start(awh[:], awh_d[:].rearrange("(k p) m -> p k m", p=128))
        vj = wp.tile([128, 4, 1], BF16, tag="vj")
        nc.sync.dma_start(vj[:], av_d[:].rearrange("(k p) o -> p k o", p=128))
        vc = wp.tile([128, 4, 1], BF16, tag="vc")
        nc.sync.dma_start(vc[:], cv_d[:].rearrange("(k p) o -> p k o", p=128))
        ab = wp.tile([128, 4, 1], F32, tag="ab")
        nc.sync.dma_start(ab[:], ab_d[:].rearrange("(k p) o -> p k o", p=128))
        cb = wp.tile([128, 4, 1], F32, tag="cb")
        nc.sync.dma_start(cb[:], cb_d[:].rearrange("(k p) o -> p k o", p=128))
        brz = wp.tile([128, 8, 1], F32, tag="brz")
        nc.sync.dma_start(brz[:], brz_d[:].rearrange("(k p) o -> p k o", p=128))
        bhn = wp.tile([128, 4, 1], F32, tag="bhn")
        nc.sync.dma_start(bhn[:], bhn_d[:].rearrange("(k p) o -> p k o", p=128))
        bin_ = wp.tile([128, 4, 1], F32, tag="bin")
        nc.sync.dma_start(bin_[:], bin_d[:].rearrange("(k p) o -> p k o", p=128))
        gw = wp.tile([128, 10, 1], BF16, tag="gw")
        nc.sync.dma_start(gw[:], gw_d[:].rearrange("(k p) o -> p k o", p=128))
        gb = wp.tile([1, 1], F32, tag="gb")
        nc.sync.dma_start(gb[:], gb_d[:])
        hTb = wp.tile([128, 4, 16], BF16, tag="hTb")
        nc.sync.dma_start(hTb[:], ht_b_d[:].rearrange("(k p) b -> p k b", p=128))
        hTf = wp.tile([128, 4, 16], F32, tag="hTf")
        nc.sync.dma_start(hTf[:], ht_f_d[:].rearrange("(k p) b -> p k b", p=128))
        ones1 = wp.tile([1, 128], BF16, tag="ones1")
        nc.vector.memset(ones1[:], 1.0)
        onesP = wp.tile([128, 1], BF16, tag="onesP")
        nc.vector.memset(onesP[:], 1.0)

        # xt = GRU input x^T [128, 10, 16]: chunks 0:2 emb, 2:10 weighted
        xt = wp.tile([128, 10, 16], BF16, tag="xt")
        nc.sync.dma_start(xt[:, 0:2, :],
                          emb_t_d[:].rearrange("(k p) b -> p k b", p=128))
        # st = state^T payload [128, 12, 16]: 0:4 h_new, 4:12 weighted
        st = wp.tile([128, 12, 16], BF16, tag="st")
        logits = wp.tile([128, VP], F32, tag="logits")

        with tc.tile_pool(name="gp", bufs=1) as gp:
            wih = gp.tile([128, 10, 1536], BF16, tag="wih")
            nc.sync.dma_start(wih[:],
                              wih_d[:].rearrange("(k p) m -> p k m", p=128))
            whh = gp.tile([128, 4, 1536], BF16, tag="whh")
            nc.sync.dma_start(whh[:],
                              whh_d[:].rearrange("(k p) m -> p k m", p=128))

            # ---------- joint attention ----------
            htj = s1.tile([128, 4, 16], F32, tag="htj")
            for mc in range(4):
                hp = psm.tile([128, 16], F32, tag="sm")
                for k in range(4):
                    nc.tensor.matmul(
                        hp[:], awh[:, k, mc * 128:(mc + 1) * 128],
                        hTb[:, k, :], start=(k == 0), stop=(k == 3))
                nc.scalar.activation(htj[:, mc, :], hp[:],
                                     mybir.ActivationFunctionType.Identity,
                                     bias=ab[:, mc, :])

            sc_j = energy_scores(awe, htj, vj, 2, 400)
            if DBG:
                nc.sync.dma_start(dbg_sc[:], sc_j[:])
            a_j = softmax_rows(sc_j, S, biasj_d)
            if DBG:
                nc.sync.dma_start(dbg_a[:], a_j[:])

            # transpose a_j -> aT [128, 4, 32] bf16 (s on partitions)
            a32 = s1.tile([32, 416], BF16, tag="a32")
            nc.vector.memset(a32[:], 0.0)
            nc.vector.tensor_copy(a32[:16, :S], a_j[:, :S])
            aT = s1.tile([128, 4, 32], BF16, tag="aT")
            for k in range(4):
                nj = 4 if k < 3 else 1
                for j32 in range(nj):
                    nc.vector.transpose(
                        aT[j32 * 32:(j32 + 1) * 32, k, 0:32],
                        a32[0:32, k * 128 + j32 * 32: k * 128 + (j32 + 1) * 32])

            # ---------- weighted sum: wT[e, b] = sum_s a[b,s] enc[s,b,e] ----------
            wT = s1.tile([128, 8, 16], F32, tag="wT")
            wd = scr_w
            for ec in range(16):
                c8, half64 = ec // 2, (ec % 2) * 64
                wps = pw.tile([1, 1024], F32, tag="wps")
                for k in range(4):
                    sl = 128 if k < 3 else 16
                    nB = s2.tile([128, 1024], BF16, tag="nB")
                    nc.sync.dma_start(
                        nB[:sl, :],
                        enc_b_d[c8, k * 128: k * 128 + sl,
                                half64:half64 + 64].rearrange(
                                    "s e j -> s (e j)"))
                    tmp = s2.tile([128, 1024], BF16, tag="wtmp")
                    nc.vector.tensor_mul(
                        tmp[:sl, :].rearrange("p (e j) -> p e j", j=16),
                        nB[:sl, :].rearrange("p (e j) -> p e j", j=16),
                        aT[:sl, k, None, 0:16].broadcast_to([sl, 64, 16]))
                    for q in range(2):
                        nc.tensor.matmul(
                            wps[0:1, q * 512:(q + 1) * 512],
                            onesP[:sl, :], tmp[:sl, q * 512:(q + 1) * 512],
                            start=(k == 0), stop=(k == 3))
                wst = s1.tile([1, 1024], F32, tag="wst")
                nc.scalar.copy(wst[:], wps[:])
                nc.sync.dma_start(
                    wd[ec * 1024:(ec + 1) * 1024].rearrange("(o n) -> o n", o=1),
                    wst[:])
            nc.sync.dma_start(
                wT[:], wd[:].rearrange("(c p j) -> p c j", p=128, j=16))
            if DBG:
                nc.sync.dma_start(dbg_wt[:], wT[:].rearrange("p c j -> p (c j)"))
            nc.vector.tensor_copy(xt[:, 2:10, :], wT[:])
            nc.vector.tensor_copy(st[:, 4:12, :], wT[:])

            # ---------- p_gen ----------
            pgp = psm.tile([1, 16], F32, tag="sm")
            for k in range(10):
                nc.tensor.matmul(pgp[:], gw[:, k, :], xt[:, k, :],
                                 start=(k == 0), stop=(k == 9))
            pgT = s1.tile([1, 16], F32, tag="pgT")
            nc.scalar.activation(pgT[:], pgp[:],
                                 mybir.ActivationFunctionType.Sigmoid,
                                 bias=gb[:])
            pgd = scr_pg
            nc.sync.dma_start(pgd[:].rearrange("(o n) -> o n", o=1), pgT[:])
            pg_b = s1.tile([16, 1], F32, tag="pg_b")
            nc.sync.dma_start(pg_b[:], pgd[:].rearrange("(b o) -> b o", o=1))

            # ---------- GRU ----------
            rz = s1.tile([128, 8, 16], F32, tag="rz")
            hnewT = s1.tile([128, 4, 16], F32, tag="hnewT")
            for mc in range(8):
                gpsum = psm.tile([128, 16], F32, tag="sm")
                for k in range(10):
                    nc.tensor.matmul(
                        gpsum[:], wih[:, k, mc * 128:(mc + 1) * 128],
                        xt[:, k, :], start=(k == 0), stop=False)
                for k in range(4):
                    nc.tensor.matmul(
                        gpsum[:], whh[:, k, mc * 128:(mc + 1) * 128],
                        hTb[:, k, :], start=False, stop=(k == 3))
                nc.scalar.activation(rz[:, mc, :], gpsum[:],
                                     mybir.ActivationFunctionType.Sigmoid,
                                     bias=brz[:, mc, :])
            for mc in range(4):
                m8 = mc + 8
                gx = psm.tile([128, 16], F32, tag="sm")
                for k in range(10):
                    nc.tensor.matmul(
                        gx[:], wih[:, k, m8 * 128:(m8 + 1) * 128],
                        xt[:, k, :], start=(k == 0), stop=(k == 9))
                gh = psm.tile([128, 16], F32, tag="sm")
                for k in range(4):
                    nc.tensor.matmul(
                        gh[:], whh[:, k, m8 * 128:(m8 + 1) * 128],
                        hTb[:, k, :], start=(k == 0), stop=(k == 3))
                hnb = s1.tile([128, 16], F32, tag="hnb")
                nc.scalar.activation(hnb[:], gh[:],
                                     mybir.ActivationFunctionType.Identity,
                                     bias=bhn[:, mc, :])
                rhn = s1.tile([128, 16], F32, tag="rhn")
                nc.vector.tensor_mul(rhn[:], rz[:, mc, :], hnb[:])
                xnr = s1.tile([128, 16], F32, tag="xnr")
                nc.vector.tensor_add(xnr[:], gx[:], rhn[:])
                n_sb = s1.tile([128, 16], F32, tag="n_sb")
                nc.scalar.activation(n_sb[:], xnr[:],
                                     mybir.ActivationFunctionType.Tanh,
                                     bias=bin_[:, mc, :])
                dh = s1.tile([128, 16], F32, tag="dh")
                nc.vector.tensor_sub(dh[:], hTf[:, mc, :], n_sb[:])
                zd = s1.tile([128, 16], F32, tag="zd")
                nc.vector.tensor_mul(zd[:], rz[:, mc + 4, :], dh[:])
                nc.vector.tensor_add(hnewT[:, mc, :], n_sb[:], zd[:])
                nc.vector.tensor_copy(st[:, mc, :], hnewT[:, mc, :])

        if DBG:
            nc.sync.dma_start(dbg_hn[:], hnewT[:].rearrange("p c j -> p (c j)"))
        # h_new natural layout -> out_h
        hp32 = s1.tile([128, 32], F32, tag="hp32")
        nc.vector.memset(hp32[:], 0.0)
        hnat = s1.tile([32, DEC], F32, tag="hnat")
        for mc in range(4):
            nc.vector.tensor_copy(hp32[:, 0:16], hnewT[:, mc, :])
            for pb in range(4):
                nc.vector.transpose(
                    hnat[0:32, mc * 128 + pb * 32: mc * 128 + (pb + 1) * 32],
                    hp32[pb * 32:(pb + 1) * 32, 0:32])
        nc.sync.dma_start(out_h[:], hnat[:16, :])

        # ---------- AG1: state ----------
        nc.sync.dma_start(
            ag1_in[:].rearrange("(p c j) -> p (c j)", p=128, j=16), st[:])
        if NO_CC:
            nc.sync.dma_start(ag1_out[0:F * BC], ag1_in[:])
        else:
            nc.gpsimd.collective_compute(
                "AllGather", mybir.AluOpType.bypass, replica_groups=rg,
                ins=[ag1_in[:]], outs=[ag1_out[:]])

        # stage-2 pools: reuse the SBUF freed by the GRU-weight pool
        es = contextlib.ExitStack()
        s3 = es.enter_context(tc.tile_pool(name="s3", bufs=1))
        s4 = es.enter_context(tc.tile_pool(name="s4", bufs=2))
        delta = s3.tile([128, VP], F32, tag="delta")

        # ---------- copy attention ----------
        cwe = wp.tile([128, 8, 512], BF16, tag="we")
        nc.sync.dma_start(cwe[:], cwe_d[:].rearrange("(k p) m -> p k m", p=128))
        cwh = wp.tile([128, 4, 512], BF16, tag="wh")
        nc.sync.dma_start(cwh[:], cwh_d[:].rearrange("(k p) m -> p k m", p=128))
        htc = s1.tile([128, 4, 16], F32, tag="htc")
        for mc in range(4):
            hp = psm.tile([128, 16], F32, tag="sm")
            for k in range(4):
                nc.tensor.matmul(
                    hp[:], cwh[:, k, mc * 128:(mc + 1) * 128],
                    st[:, k, :], start=(k == 0), stop=(k == 3))
            nc.scalar.activation(htc[:, mc, :], hp[:],
                                 mybir.ActivationFunctionType.Identity,
                                 bias=cb[:, mc, :])
        sc_c = energy_scores(cwe, htc, vc, 1, 200)
        a_c = softmax_rows(sc_c, S1, biasc_d)
        adp = s1.tile([16, 201], F32, tag="adp")
        apg = s1.tile([16, S1], F32, tag="apg")
        nc.vector.tensor_scalar_mul(apg[:], a_c[:, :S1], pg_b[:])
        nc.vector.tensor_sub(adp[:, 0:S1], a_c[:, :S1], apg[:])
        nc.vector.tensor_copy(adp[:, 200:201], pg_b[:])
        nc.sync.dma_start(
            ag2_in[:].rearrange("(b c) -> b c", b=16), adp[:])
        if NO_CC:
            nc.sync.dma_start(ag2_out[0:BC * 201], ag2_in[:])
        else:
            nc.gpsimd.collective_compute(
                "AllGather", mybir.AluOpType.bypass, replica_groups=rg,
                ins=[ag2_in[:]], outs=[ag2_out[:]])

        # ---------- stage 2: fc matmul over vocab shard ----------
        st2 = s1.tile([128, 12, 128], BF16, tag="st2")
        nc.sync.dma_start(
            st2[:], ag1_out[:].rearrange("(r p c j) -> p c r j",
                                         r=NCORES, p=128, j=16))
        if DBG:
            st2f = s1.tile([128, 12, 128], F32, tag="st2f")
            nc.vector.tensor_copy(st2f[:], st2[:])
            nc.sync.dma_start(dbg_st2[:], st2f[:].rearrange("p c j -> p (c j)"))
        if NO_FC:
            nc.vector.memset(logits[:], 0.001)
        v0 = 0
        while v0 < (VP if not NO_FC else 0):
            nv = min(512, VP - v0)
            fw = s4.tile([128, 12, 512], BF16, tag="fw")
            nc.sync.dma_start(
                fw[:, :, :nv],
                fcw_d[:, v0:v0 + nv].rearrange("(k p) n -> p k n", p=128))
            fcbt = s4.tile([1, 512], BF16, tag="fcb_t")
            nc.sync.dma_start(fcbt[0:1, :nv], fcb_d[0:1, v0:v0 + nv])
            lp = pmm.tile([128, 512], F32, tag="mm")
            for k in range(12):
                nc.tensor.matmul(lp[:, :nv], st2[:, k, :], fw[:, k, :nv],
                                 start=(k == 0), stop=False)
            nc.tensor.matmul(lp[:, :nv], ones1[:], fcbt[0:1, :nv],
                             start=False, stop=True)
            nc.scalar.copy(logits[:, v0:v0 + nv], lp[:, :nv])
            v0 += nv

        if DBG:
            nc.sync.dma_start(dbg_lg[:], logits[:, 0:512])
        m_c = s1.tile([128, 1], F32, tag="m_c")
        nc.vector.reduce_max(m_c[:], logits[:], axis=mybir.AxisListType.X)
        nmc = s1.tile([128, 1], F32, tag="nmc")
        nc.vector.tensor_scalar_mul(nmc[:], m_c[:], -1.0)
        nc.scalar.activation(logits[:], logits[:],
                             mybir.ActivationFunctionType.Exp, bias=nmc[:])
        s_c = s1.tile([128, 1], F32, tag="s_c")
        nc.vector.reduce_sum(s_c[:], logits[:], axis=mybir.AxisListType.X)

        stats = s1.tile([128, 2], F32, tag="stats")
        nc.vector.tensor_copy(stats[:, 0:1], m_c[:])
        nc.vector.tensor_copy(stats[:, 1:2], s_c[:])
        nc.sync.dma_start(
            ag3_in[:].rearrange("(p c) -> p c", p=128), stats[:])
        if NO_CC:
            nc.sync.dma_start(ag3_out[0:B * 2], ag3_in[:])
        else:
            nc.gpsimd.collective_compute(
                "AllGather", mybir.AluOpType.bypass, replica_groups=rg,
                ins=[ag3_in[:]], outs=[ag3_out[:]])

        mg = s1.tile([128, 8, 2], F32, tag="mg")
        nc.sync.dma_start(
            mg[:], ag3_out[:].rearrange("(r p c) -> p r c", r=NCORES, p=128))
        M = s1.tile([128, 1], F32, tag="M")
        nc.vector.reduce_max(M[:], mg[:, :, 0:1], axis=mybir.AxisListType.XY)
        nM = s1.tile([128, 1], F32, tag="nM")
        nc.vector.tensor_scalar_mul(nM[:], M[:], -1.0)
        eR = s1.tile([128, 8], F32, tag="eR")
        nc.scalar.activation(eR[:], mg[:, :, 0],
                             mybir.ActivationFunctionType.Exp, bias=nM[:])
        t8 = s1.tile([128, 8], F32, tag="t8")
        nc.vector.tensor_mul(t8[:], eR[:], mg[:, :, 1])
        Ssum = s1.tile([128, 1], F32, tag="Ssum")
        nc.vector.reduce_sum(Ssum[:], t8[:], axis=mybir.AxisListType.X)
        Sinv = s1.tile([128, 1], F32, tag="Sinv")
        nc.vector.reciprocal(Sinv[:], Ssum[:])
        emd = s1.tile([128, 1], F32, tag="emd")
        nc.scalar.activation(emd[:], m_c[:],
                             mybir.ActivationFunctionType.Exp, bias=nM[:])
        pgf = s1.tile([128, 1], F32, tag="pgf")
        nc.sync.dma_start(
            pgf[:], ag2_out[:].rearrange("(b c) -> b c", c=201)[:, 200:201])
        alpha = s1.tile([128, 1], F32, tag="alpha")
        nc.vector.tensor_mul(alpha[:], emd[:], Sinv[:])
        nc.vector.tensor_mul(alpha[:], alpha[:], pgf[:])

        # ---------- scatter_add of copy probabilities ----------
        adf = s1.tile([128, 200], F32, tag="adf")
        nc.sync.dma_start(
            adf[:], ag2_out[:].rearrange("(b c) -> b c", c=201)[:, 0:200])
        if DBG:
            nc.sync.dma_start(dbg_adf[:], adf[:])
            dal = s1.tile([128, 4], F32, tag="dal")
            nc.vector.tensor_copy(dal[:, 0:1], alpha[:])
            nc.vector.tensor_copy(dal[:, 1:2], m_c[:])
            nc.vector.tensor_copy(dal[:, 2:3], s_c[:])
            nc.vector.tensor_copy(dal[:, 3:4], Ssum[:])
            nc.sync.dma_start(dbg_al[:], dal[:])
        for pc in range(NPIECE if not NO_SCAT else 0):
            for r in range(R):
                ix = s2.tile([128, 512], I16, tag="idx_t")
                nc.sync.dma_start(
                    ix[:],
                    idx_d[(pc * R + r) * 128:(pc * R + r + 1) * 128, :])
                dl = s2.tile([128, PW], F32, tag="dl")
                nc.gpsimd.local_scatter(
                    dl[:].bitcast(U16), adf[:].bitcast(U16), ix[:, 0:400],
                    channels=128, num_elems=2 * PW, num_idxs=400)
                if r == 0:
                    nc.vector.tensor_copy(delta[:, pc * PW:(pc + 1) * PW], dl[:])
                else:
                    nc.vector.tensor_add(delta[:, pc * PW:(pc + 1) * PW],
                                         delta[:, pc * PW:(pc + 1) * PW],
                                         dl[:])
        nc.vector.scalar_tensor_tensor(
            out=logits[:], in0=logits[:], scalar=alpha[:], in1=delta[:],
            op0=mybir.AluOpType.mult, op1=mybir.AluOpType.add)
        nc.sync.dma_start(out_v[:], logits[:, 0:64] if SMALL_OUT else logits[:])
        es.close()

    nc.compile()
    return nc


def _prep(inputs):
    """Host-side sharding/layout prep. Returns (in_maps, R)."""
    inp = {k: np.asarray(v) for k, v in inputs.items()}
    e1, e2 = inp["encoder_outputs1"], inp["encoder_outputs2"]
    src1 = inp["src1"].astype(np.int64)
    emb_all = inp["embedding"][inp["input"].astype(np.int64)]  # [B, EMB]
    attn_W, attn_b, attn_v = inp["attn_W"], inp["attn_b"], inp["attn_v"]
    copy_W, copy_b, copy_v = inp["copy_W"], inp["copy_b"], inp["copy_v"]
    mask_j = np.concatenate([inp["mask1"], inp["mask2"]], axis=1)
    mask_c = inp["mask1"] * inp["triple_mask"]
    bias_j_all = np.where(mask_j == 0, NEG, 0.0).astype(np.float32)
    bias_c_all = np.where(mask_c == 0, NEG, 0.0).astype(np.float32)

    shared = {
        "aWeT": attn_W[DEC:].astype(BF), "aWhT": np.ascontiguousarray(attn_W[:DEC]).astype(BF),
        "ab": attn_b[:, None].astype(np.float32), "av": attn_v[:, None].astype(BF),
        "cWeT": copy_W[DEC:].astype(BF), "cWhT": np.ascontiguousarray(copy_W[:DEC]).astype(BF),
        "cb": copy_b[:, None].astype(np.float32), "cv": copy_v[:, None].astype(BF),
        "WihT": np.ascontiguousarray(inp["gru_Wih"].T).astype(BF),
        "WhhT": np.ascontiguousarray(inp["gru_Whh"].T).astype(BF),
        "brz": (inp["gru_bih"] + inp["gru_bhh"])[:2 * DEC, None].astype(np.float32),
        "bhn": inp["gru_bhh"][2 * DEC:, None].astype(np.float32),
        "bin": inp["gru_bih"][2 * DEC:, None].astype(np.float32),
        "gw": np.ascontiguousarray(
            np.concatenate([inp["gate_W"][0, E2:], inp["gate_W"][0, :E2]])[:, None]).astype(BF),
        "gb": inp["gate_b"].reshape(1, 1).astype(np.float32),
    }

    # scatter plan: occurrence rounds
    occ = {}
    rmax = 1
    entry = [[] for _ in range(NCORES)]
    for s in range(S1):
        for b in range(B):
            v = int(src1[s, b])
            c = min(v // VC, NCORES - 1)
            vl = v - c * VC
            key = (b, v)
            r = occ.get(key, 0)
            occ[key] = r + 1
            rmax = max(rmax, r + 1)
            entry[c].append((b, s, vl, r))
    R = rmax

    in_maps = []
    for c in range(NCORES):
        rows = slice(c * BC, (c + 1) * BC)
        enc = np.concatenate([e1[:, rows], e2[:, rows]], axis=0).astype(BF)
        encT = np.ascontiguousarray(enc.transpose(2, 1, 0)).reshape(E2, JROWS)
        encB = np.ascontiguousarray(
            enc.reshape(S, BC, 8, 128).transpose(2, 0, 3, 1))
        hT = np.ascontiguousarray(inp["hidden"][rows].T)
        fcw = inp["fc_W"][c * VC:(c + 1) * VC]
        fcwT = np.zeros((F, VP), dtype=BF)
        fcwT[:, :VC] = np.ascontiguousarray(fcw.T).astype(BF)
        fcb = np.full((1, VP), -30000.0, dtype=BF)
        fcb[0, :VC] = inp["fc_b"][c * VC:(c + 1) * VC].astype(BF)
        idxt = np.full((NPIECE, R, 128, 512), -1, dtype=np.int16)
        for (b, s, vl, r) in entry[c]:
            pc, off = vl // PW, vl % PW
            idxt[pc, r, b, 2 * s] = 2 * off
            idxt[pc, r, b, 2 * s + 1] = 2 * off + 1
        m = {
            "encT": encT,
            "encB": encB,
            "hTf": hT.astype(np.float32),
            "hTb": hT.astype(BF),
            "embT": np.ascontiguousarray(emb_all[rows].T).astype(BF),
            "biasJ": bias_j_all[rows],
            "biasC": bias_c_all[rows],
            "fcWT": fcwT,
            "fcb": fcb,
            "idxT": idxt.reshape(NPIECE * R * 128, 512),
        }
        m.update(shared)
        in_maps.append(m)
    return in_maps, R


_NC_CACHE = {}


def kernel(**inputs):
    in_maps, R = _prep(inputs)
    if R not in _NC_CACHE:
        _NC_CACHE[R] = build_nc(R)
    nc = _NC_CACHE[R]
    res = bass_utils.run_bass_kernel_spmd(nc, in_maps,
                                          core_ids=list(range(NCORES)))
    final = np.concatenate([res.results[c]["out_v"][:, :VC]
                            for c in range(NCORES)], axis=1)[:, :V]
    h_new = np.concatenate([res.results[c]["out_h"]
                            for c in range(NCORES)], axis=0)
    return final.astype(np.float32), h_new.astype(np.float32)
